# revision 1
# baseline (speedup 1.0000x reference)
"""DCNv3 (deformable conv v3) forward as a Bass/Tile kernel for Trainium2.

Contract: kernel(**inputs) takes the FULL inputs of reference.setup_inputs()
and returns the FULL (8, 64, 64, 128) output. The batch dim (8) is
data-parallel across 8 NeuronCores; each core runs an identical single-image
program (no collectives).

Algorithm (validated vs the jax reference in numpy, rel err ~4e-6):
  x_proj = x @ w_in + b_in
  x1     = gelu(LN(dwconv3x3(x) + dw_b) * ln_g + ln_b)
  offs   = x1 @ w_off + b_off        (per group g, point p: (ox, oy), |o|<1)
  e      = exp(x1 @ w_msk + b_msk);  m = e / sum_p e
  Bilinear sampling of point p at (h+1+ky+oy, w+1+kx+ox) decomposes into
  per-axis 3-tap tents  t[-1]=relu(-o), t[0]=1-|o|, t[1]=relu(o), so the
  mask-weighted sample sum collapses to a 5x5 shift window:
     out[pos, (g,c)] = sum_{sy,sx in [-2,2]} A[(g,sy,sx), pos] *
                       xproj_pad2[pos + (sy,sx), (g,c)]
  with A[(g,s)] = sum_p m_p * ty_p(sy-ky) * tx_p(sx-kx).  xproj is padded by
  2 (inner ring = conv pad inside the sampling grid, outer ring = zeros =
  grid_sample zero padding), making all window reads in-bounds with no
  boundary special cases.
  final  = out @ w_out + b_out

Layout: channel-major [C on partitions, positions on the free axis], so all
channel contractions are natural matmuls. Matmul operands are bf16 (full PE
rate + FWL weight loads); all accumulation (PSUM) is fp32. The output
projection is fused into the apply loop: final = sum_s (A_s ⊙ img_s) @ w_out
accumulates across the 25 shifts in PSUM, so the shift loop needs no
elementwise adds at all; the pos-major result is produced by
identity-matmul transposes. A-build and apply phases share the schedule
(two concurrently-open PSUM pools) so their dependency bubbles overlap.
"""

from contextlib import ExitStack

import ml_dtypes
import numpy as np

import concourse.bass as bass
import concourse.mybir as mybir
import concourse.tile as tile
from concourse._compat import with_exitstack

N, H, W, C, G, K = 8, 64, 64, 128, 8, 3
GC = C // G            # 16
P = K * K              # 9
POS = H * W            # 4096
HP, WP = H + 2, W + 2            # dwconv pad-1 grid (66)
HP2, WP2 = H + 4, W + 4          # sampling pad-2 grid (68)
EPS = 1e-6
NS = 25                          # 5x5 shift window
NH1, NH2 = 13, 12                # A row split: s in [0,13), [13,25)
R1, R2 = G * NH1, G * NH2        # 104, 96 partition rows of the two A halves
F32 = mybir.dt.float32
BF16 = mybir.dt.bfloat16
NPBF = ml_dtypes.bfloat16

CHUNK = 512                      # free-dim chunk for the build phase
NCH = POS // CHUNK               # 8
ACH = 1024                       # free-dim chunk for the apply phase
NACH = POS // ACH                # 4

AF = mybir.ActivationFunctionType
OP = mybir.AluOpType


# --------------------------------------------------------------------------
# host-side constant matrices
# --------------------------------------------------------------------------

def _host_constants(inputs):
    dw_w = np.asarray(inputs["dw_w"], np.float32)        # (3,3,1,C) [ky,kx]
    w_off = np.asarray(inputs["w_off"], np.float32)      # (C, G*P*2)
    b_off = np.asarray(inputs["b_off"], np.float32)      # (G*P*2,)

    # depthwise weights as 9 diagonal matrices, c-major: [c_row, s, c_col]
    dwdiag = np.zeros((C, P, C), np.float32)
    for s in range(P):
        ky, kx = s // 3, s % 3
        dwdiag[np.arange(C), s, np.arange(C)] = dw_w[ky, kx, 0]

    w_offx = np.ascontiguousarray(w_off[:, 0::2])        # (C, 72)
    w_offy = np.ascontiguousarray(w_off[:, 1::2])
    b_offx = np.ascontiguousarray(b_off[0::2])           # (72,)
    b_offy = np.ascontiguousarray(b_off[1::2])

    # group-sum / group-broadcast 0/1 matrices for the mask softmax
    eg = np.zeros((G * P, G), np.float32)                # lhsT: (72 gp, 8 g)
    egt = np.zeros((G, G * P), np.float32)               # lhsT: (8 g, 72 gp)
    for g in range(G):
        eg[g * P:(g + 1) * P, g] = 1.0
        egt[g, g * P:(g + 1) * P] = 1.0

    # A-scatter matrices: contribution (g,p) of term (dy,dx) lands in
    # A row (g, s) with s = (p%3 + dy)*5 + (p//3 + dx)   (x-major p!)
    m1 = np.zeros((G * P, 9, R1), np.float32)
    m2 = np.zeros((G * P, 9, R2), np.float32)
    for dy in range(3):
        for dx in range(3):
            d = dy * 3 + dx
            for g in range(G):
                for p in range(P):
                    s = (p % 3 + dy) * 5 + (p // 3 + dx)
                    # tent factors for d-index 0 ("-1" tap) and 1 ("0" tap)
                    # are stored negated on-chip; correct the sign here
                    sgn = (-1.0 if dy < 2 else 1.0) * (-1.0 if dx < 2 else 1.0)
                    if s < NH1:
                        m1[g * P + p, d, g * NH1 + s] = sgn
                    else:
                        m2[g * P + p, d, g * NH2 + (s - NH1)] = sgn

    # A-broadcast matrices: A row (g, s) -> output row (g*GC + c)
    ea1 = np.zeros((R1, NH1, C), np.float32)
    ea2 = np.zeros((R2, NH2, C), np.float32)
    for g in range(G):
        for sl in range(NH1):
            ea1[g * NH1 + sl, sl, g * GC:(g + 1) * GC] = 1.0
        for sl in range(NH2):
            ea2[g * NH2 + sl, sl, g * GC:(g + 1) * GC] = 1.0

    ones_row = np.ones((1, C), np.float32)
    invc_col = np.full((C, 1), 1.0 / C, np.float32)
    i128 = np.eye(C, dtype=np.float32)
    b_out_row = np.asarray(inputs["b_out"], np.float32).reshape(1, C)
    ones_n = np.ones((1, 512), np.float32)

    return {
        "dwdiag": dwdiag, "w_offx": w_offx, "w_offy": w_offy,
        "b_offx": b_offx, "b_offy": b_offy,
        "b_offxn": -b_offx, "b_offyn": -b_offy,
        "eg": eg, "egt": egt,
        "m1": m1, "m2": m2, "ea1": ea1, "ea2": ea2,
        "ones_row": ones_row, "invc_col": invc_col,
        "i128": i128, "b_out_row": b_out_row, "ones_n": ones_n,
    }


# names of DRAM inputs delivered as bf16 (matmul operands; x additionally
# rides the DMA xbar transpose, which needs a 2-byte dtype).
_BF16_INPUTS = {
    "x", "w_in", "dwdiag", "w_offx", "w_offy", "w_msk", "w_out",
    "eg", "egt", "m1", "m2", "ea1", "ea2", "ones_row", "invc_col",
    "i128", "b_out_row", "ones_n",
}


# --------------------------------------------------------------------------
# the per-core Tile program
# --------------------------------------------------------------------------

@with_exitstack
def _dcn_tile(ctx: ExitStack, tc: tile.TileContext, io: dict):
    nc = tc.nc
    ctx.enter_context(nc.allow_low_precision(
        reason="bf16 matmul operands; accumulation stays fp32 in PSUM and "
               "in the fp32 sampling accumulator"))

    persist = ctx.enter_context(tc.tile_pool(name="persist", bufs=1))
    temps = ctx.enter_context(tc.tile_pool(name="temps", bufs=3))
    tents = ctx.enter_context(tc.tile_pool(name="tents", bufs=2))

    # ---- load weights / constants -------------------------------------
    def load(name, shape, col=False):
        dt = BF16 if name in _BF16_INPUTS else F32
        t = persist.tile(shape, dt, tag=f"w_{name}")
        src = io[name]
        if col:  # DRAM vector (n,) -> SBUF [n, 1]
            src = bass.AP(tensor=src.tensor, offset=src.offset,
                          ap=[[1, shape[0]], [1, 1]])
        nc.sync.dma_start(out=t, in_=src)
        return t

    w_in = load("w_in", [C, C])
    w_out = load("w_out", [C, C])
    w_offx = load("w_offx", [C, G * P])
    w_offy = load("w_offy", [C, G * P])
    w_msk = load("w_msk", [C, G * P])
    dwdiag = load("dwdiag", [C, P, C])          # [c_row, s, c_col]
    eg = load("eg", [G * P, G])
    egt = load("egt", [G, G * P])
    m1 = load("m1", [G * P, 9, R1])
    m2 = load("m2", [G * P, 9, R2])
    ea1 = load("ea1", [R1, NH1, C])
    ea2 = load("ea2", [R2, NH2, C])
    ones_row = load("ones_row", [1, C])
    invc_col = load("invc_col", [C, 1])
    i128 = load("i128", [C, C])
    b_out_row = load("b_out_row", [1, C])
    ones_n = load("ones_n", [1, CHUNK])
    b_in = load("b_in", [C, 1], col=True)
    dw_b = load("dw_b", [C, 1], col=True)
    ln_g = load("ln_g", [C, 1], col=True)
    ln_b = load("ln_b", [C, 1], col=True)
    b_offx = load("b_offx", [G * P, 1], col=True)
    b_offy = load("b_offy", [G * P, 1], col=True)
    b_offxn = load("b_offxn", [G * P, 1], col=True)
    b_offyn = load("b_offyn", [G * P, 1], col=True)
    b_msk = load("b_msk", [G * P, 1], col=True)

    eps1 = persist.tile([1, 1], F32)
    nc.vector.memset(eps1, EPS)

    # ---- persistent activations ---------------------------------------
    xt_pad = persist.tile([C, HP, WP], BF16)     # x^T, conv-padded (66x66)
    xpj_pad = persist.tile([C, HP2, WP2], BF16)  # x_proj^T, pad-2 (68x68)
    a1 = [persist.tile([R1, CHUNK], BF16, tag=f"a1_{i}", name=f"a1_{i}")
          for i in range(NCH)]
    a2 = [persist.tile([R2, CHUNK], BF16, tag=f"a2_{i}", name=f"a2_{i}")
          for i in range(NCH)]

    nc.vector.memset(xt_pad, 0.0)
    nc.vector.memset(xpj_pad, 0.0)

    # ---- stage 1+2: transpose x in (matmul against identity);
    #      x_proj into the pad-2 grid ----
    with tc.tile_pool(name="ps_s12", bufs=2, space="PSUM") as psum:
        for piece in range(4):       # 8 pos-tiles = 16 h-rows per piece
            x_pm = temps.tile([C, 8, C], BF16, tag="x_pm")
            nc.sync.dma_start(
                out=x_pm,
                in_=bass.AP(tensor=io["x"].tensor,
                            offset=io["x"].offset + piece * 8 * C * C,
                            ap=[[C, C], [C * C, 8], [1, C]]))
            for half in range(2):    # 4 pos-tiles -> one [C, 512] psum
                pst = psum.tile([C, 4, C], F32, tag="pst")
                for q in range(4):
                    tt = half * 4 + q
                    # out[c, pos] = sum_p x[p, c] * I[p, pos-col]
                    nc.tensor.matmul(pst[:, q, :], x_pm[:, tt, :], i128,
                                     start=True, stop=True)
                t0 = piece * 16 + half * 8    # first h-row of this group
                nc.scalar.copy(
                    out=xt_pad[:, 1 + t0:1 + t0 + 8, 1:1 + W],
                    in_=pst.rearrange("c a (b d) -> c (a b) d", d=W))

        for ch in range(NCH):        # chunk = 8 h-rows
            h0 = ch * 8
            rhs = xt_pad[:, 1 + h0:1 + h0 + 8, 1:1 + W]
            ps = psum.tile([C, CHUNK], F32, tag="ps_proj")
            nc.tensor.matmul(ps, w_in, rhs, start=True, stop=True)
            nc.scalar.activation(
                out=xpj_pad[:, 2 + h0:2 + h0 + 8, 2:2 + W],
                in_=ps.rearrange("c (a b) -> c a b", b=W),
                func=AF.Identity, bias=b_in, scale=1.0)

    # ---- stages 3+4, interleaved ---------------------------------------
    # A 1-element-shifted copy of xpj_pad keeps odd-sx window reads
    # 4B-aligned so the apply muls hit the DVE 2x bf16 mode.
    xpj_odd = persist.tile([C, HP2, WP2], BF16)
    xpjf = xpj_pad.rearrange("c a b -> c (a b)")
    xpjof = xpj_odd.rearrange("c a b -> c (a b)")
    nc.scalar.copy(out=xpjof[:, :HP2 * WP2 - 1], in_=xpjf[:, 1:])

    # Two PSUM pools, both open across the whole fused phase so the
    # scheduler can interleave A-building and applying freely:
    #   ps3 (6 banks): psmm x2, psst x1, psoff x1, psa1 x1, psa2 x1
    #   ps4 (2 banks): psoacc x1, psab x1 (psot shares the psab tag)
    with tc.tile_pool(name="ps3", bufs=1, space="PSUM") as psum, \
            tc.tile_pool(name="ps4", bufs=1, space="PSUM") as psum4:

        def build_chunk(ch):
            h0 = ch * 8
            # depthwise conv via 9 diagonal matmuls
            psA = psum.tile([C, CHUNK], F32, tag="psmm", bufs=1, name="psA")
            for s in range(P):
                ky, kx = s // 3, s % 3
                rhs = xt_pad[:, h0 + ky:h0 + ky + 8, kx:kx + W]
                nc.tensor.matmul(psA, dwdiag[:, s, :], rhs,
                                 start=(s == 0), stop=(s == P - 1))
            x1c = temps.tile([C, CHUNK], BF16, tag="x1c")
            nc.scalar.activation(out=x1c, in_=psA, func=AF.Identity,
                                 bias=dw_b, scale=1.0)

            # LayerNorm over channels (partition dim) via 1/C col matmuls
            x1sq = temps.tile([C, CHUNK], BF16, tag="scr")
            nc.vector.tensor_mul(out=x1sq, in0=x1c, in1=x1c)
            psm = psum.tile([1, CHUNK], F32, tag="psst", bufs=1, name="psm")
            nc.tensor.matmul(psm, invc_col, x1c, start=True, stop=True)
            mean_r = temps.tile([1, CHUNK], BF16, tag="mean_r")
            nc.scalar.copy(out=mean_r, in_=psm)
            psq = psum.tile([1, CHUNK], F32, tag="psst", bufs=1, name="psq")
            nc.tensor.matmul(psq, invc_col, x1sq, start=True, stop=True)
            tmp_r = temps.tile([1, CHUNK], F32, tag="tmp_r")
            nc.scalar.activation(out=tmp_r, in_=mean_r, func=AF.Square)
            nc.vector.tensor_sub(out=tmp_r, in0=psq, in1=tmp_r)   # var
            nc.scalar.activation(out=tmp_r, in_=tmp_r, func=AF.Sqrt,
                                 bias=eps1, scale=1.0)            # std
            rstd_r = temps.tile([1, CHUNK], BF16, tag="rstd_r")
            nc.vector.reciprocal(out=rstd_r, in_=tmp_r)           # rstd
            mrs_r = temps.tile([1, CHUNK], BF16, tag="mrs_r")
            nc.vector.tensor_mul(out=mrs_r, in0=mean_r, in1=rstd_r)
            psR = psum.tile([C, CHUNK], F32, tag="psmm", bufs=1, name="psR")
            nc.tensor.matmul(psR, ones_row, rstd_r, start=True, stop=True)
            zc = temps.tile([C, CHUNK], F32, tag="scr2")
            nc.vector.tensor_mul(out=zc, in0=x1c, in1=psR)
            psM = psum.tile([C, CHUNK], F32, tag="psmm", bufs=1, name="psM")
            nc.tensor.matmul(psM, ones_row, mrs_r, start=True, stop=True)
            nc.vector.tensor_sub(out=zc, in0=zc, in1=psM)
            x1gc = temps.tile([C, CHUNK], BF16, tag="x1gc")
            nc.scalar.activation(out=x1gc, in_=zc, func=AF.Gelu,
                                 bias=ln_b, scale=ln_g)

            # offset / mask projections + tents (tents on DVE; the d=0/1
            # tent factors are stored negated, signs baked into m1/m2):
            #   txmn = min(0, o);  txp = relu(o);  tx0n = txp - 1 - txmn
            psX = psum.tile([G * P, CHUNK], F32, tag="psoff", bufs=1,
                            name="psX")
            nc.tensor.matmul(psX, w_offx, x1gc, start=True, stop=True)
            txmn = tents.tile([G * P, CHUNK], BF16, tag="txmn")
            txp = tents.tile([G * P, CHUNK], BF16, tag="txp")
            tx0n = tents.tile([G * P, CHUNK], BF16, tag="tx0n")
            nc.vector.tensor_scalar(out=txmn, in0=psX, scalar1=b_offx,
                                    scalar2=0.0, op0=OP.add, op1=OP.min)
            nc.vector.tensor_scalar(out=txp, in0=psX, scalar1=b_offx,
                                    scalar2=0.0, op0=OP.add, op1=OP.max)
            nc.vector.scalar_tensor_tensor(out=tx0n, in0=txp, scalar=-1.0,
                                           in1=txmn, op0=OP.add,
                                           op1=OP.subtract)
            psY = psum.tile([G * P, CHUNK], F32, tag="psoff", bufs=1,
                            name="psY")
            nc.tensor.matmul(psY, w_offy, x1gc, start=True, stop=True)
            tymn = tents.tile([G * P, CHUNK], BF16, tag="tymn")
            typ = tents.tile([G * P, CHUNK], BF16, tag="typ")
            ty0n = tents.tile([G * P, CHUNK], BF16, tag="ty0n")
            nc.vector.tensor_scalar(out=tymn, in0=psY, scalar1=b_offy,
                                    scalar2=0.0, op0=OP.add, op1=OP.min)
            nc.vector.tensor_scalar(out=typ, in0=psY, scalar1=b_offy,
                                    scalar2=0.0, op0=OP.add, op1=OP.max)
            nc.vector.scalar_tensor_tensor(out=ty0n, in0=typ, scalar=-1.0,
                                           in1=tymn, op0=OP.add,
                                           op1=OP.subtract)

            # normalized mask: e_n = exp(l + b) / group sum
            psE = psum.tile([G * P, CHUNK], F32, tag="psoff", bufs=1,
                            name="psE")
            nc.tensor.matmul(psE, w_msk, x1gc, start=True, stop=True)
            ec = temps.tile([G * P, CHUNK], BF16, tag="ec")
            nc.scalar.activation(out=ec, in_=psE, func=AF.Exp,
                                 bias=b_msk, scale=1.0)
            psS = psum.tile([G, CHUNK], F32, tag="psoff", bufs=1,
                            name="psS")
            nc.tensor.matmul(psS, eg, ec, start=True, stop=True)
            s_r = temps.tile([G, CHUNK], BF16, tag="s_r")
            nc.vector.reciprocal(out=s_r, in_=psS)
            psB = psum.tile([G * P, CHUNK], F32, tag="psoff", bufs=1,
                            name="psB")
            nc.tensor.matmul(psB, egt, s_r, start=True, stop=True)
            nc.vector.tensor_mul(out=ec, in0=ec, in1=psB)

            # A scatter: 9 (dy,dx) terms, each = (e_n * ty_dy) * tx_dx
            psA1 = psum.tile([R1, CHUNK], F32, tag="psa1", bufs=1,
                             name="psA1")
            psA2 = psum.tile([R2, CHUNK], F32, tag="psa2", bufs=1,
                             name="psA2")
            tys = [tymn, ty0n, typ]
            txs = [txmn, tx0n, txp]
            for dy in range(3):
                mty = temps.tile([G * P, CHUNK], BF16, tag="mty")
                nc.vector.tensor_mul(out=mty, in0=ec, in1=tys[dy])
                for dx in range(3):
                    d = dy * 3 + dx
                    ctr = temps.tile([G * P, CHUNK], BF16, tag="ctr")
                    nc.vector.tensor_mul(out=ctr, in0=mty, in1=txs[dx])
                    nc.tensor.matmul(psA1, m1[:, d, :], ctr,
                                     start=(d == 0), stop=(d == 8))
                    nc.tensor.matmul(psA2, m2[:, d, :], ctr,
                                     start=(d == 0), stop=(d == 8))
            nc.scalar.copy(out=a1[ch], in_=psA1)
            nc.scalar.copy(out=a2[ch], in_=psA2)

        def apply_chunk(ch):
            h0 = ch * 8            # 8 h-rows per 512-pos chunk
            outp = psum4.tile([C, CHUNK], F32, tag="psoacc", bufs=1,
                              name="outp")
            nc.tensor.matmul(outp, b_out_row, ones_n, start=True,
                             stop=False)
            for s in range(NS):
                sy, sx = s // 5 - 2, s % 5 - 2
                if s < NH1:
                    lhsT, arows = ea1[:, s, :], a1[ch]
                else:
                    lhsT, arows = ea2[:, s - NH1, :], a2[ch]
                psBc = psum4.tile([C, CHUNK], F32, tag="psab", bufs=2,
                                  name="psBc")
                nc.tensor.matmul(psBc, lhsT, arows, start=True, stop=True)
                ab = temps.tile([C, CHUNK], BF16, tag="ab")
                if False:            # all evictions on ACT
                    nc.vector.tensor_copy(out=ab, in_=psBc)
                else:
                    nc.scalar.copy(out=ab, in_=psBc)
                row = 2 + sy + h0
                if sx % 2 == 0:      # even col offset: 4B-aligned in bf16
                    img = xpj_pad[:, row:row + 8, 2 + sx:2 + sx + W]
                else:                # odd: read the 1-shifted copy
                    img = xpj_odd[:, row:row + 8, 1 + sx:1 + sx + W]
                t = temps.tile([C, CHUNK], BF16, tag="t_app")
                nc.vector.tensor_mul(out=t, in0=ab, in1=img)
                nc.tensor.matmul(outp, w_out, t, start=False,
                                 stop=(s == NS - 1))

            # evict, transpose to pos-major via identity matmuls, store
            fsb = temps.tile([C, CHUNK], BF16, tag="fsb")
            nc.scalar.copy(out=fsb, in_=outp)
            pso = psum4.tile([C, 4, C], F32, tag="psab", bufs=2, name="pso")
            for q in range(4):
                nc.tensor.matmul(pso[:, q, :], fsb[:, q * C:(q + 1) * C],
                                 i128, start=True, stop=True)
            osb = temps.tile([C, 4, C], F32, tag="osb")
            nc.scalar.copy(out=osb, in_=pso)
            pos0 = ch * CHUNK
            nc.sync.dma_start(
                out=bass.AP(tensor=io["out"].tensor,
                            offset=io["out"].offset + pos0 * C,
                            ap=[[C, C], [C * C, 4], [1, C]]),
                in_=osb)

        for ch in range(NCH):
            build_chunk(ch)
            if ch > 0:
                apply_chunk(ch - 1)
        apply_chunk(NCH - 1)


# --------------------------------------------------------------------------
# bass module build + public entry point
# --------------------------------------------------------------------------

# Hardware TPB instructions carry exactly ONE sync-wait slot (the
# NEURON_ISA_TPB_EVENTS struct).  Tile can emit several waits on one BIR
# instruction; walrus splits matmult waits across the LDWEIGHTS/MATMULT
# pair, but single-struct ops (Activation, ...) fail codegen with "Too many
# sync wait commands".  Move surplus waits onto standalone same-engine
# EventSemaphore instructions inserted immediately before the offender.
def _wait_cap(ins):
    t = type(ins).__name__
    if t == "InstEventSemaphore":
        return None
    return 1


def _split_surplus_waits(nc):
    import bass_rust
    n = 0
    for bb in nc.m.functions[0].blocks:
        out = []
        for ins in bb.instructions:
            si = getattr(ins, "sync_info", None)
            cap = _wait_cap(ins)
            if si is not None and cap is not None and len(si.on_wait) > cap:
                waits = list(si.on_wait)
                for i, w in enumerate(waits[:-cap]):
                    ev = mybir.InstEventSemaphore(
                        name=f"{ins.name}_xw{i}", ins=[], outs=[])
                    ev.engine = ins.engine
                    ev.sync_info = bass_rust.SyncInfo(on_wait=[w],
                                                     on_update=[])
                    nc.register_instruction(ev)
                    out.append(ev)
                    n += 1
                ins.sync_info = bass_rust.SyncInfo(
                    on_wait=waits[-cap:], on_update=list(si.on_update))
            out.append(ins)
        bb.instructions = out
    return n


_CACHED = {}


def _build_bass():
    if "nc" in _CACHED:
        return _CACHED["nc"]
    nc = bass.Bass()
    io = {}
    specs = {
        "x": (POS, C), "w_in": (C, C), "b_in": (C,), "dwdiag": (C, P, C),
        "dw_b": (C,), "ln_g": (C,), "ln_b": (C,),
        "w_offx": (C, G * P), "w_offy": (C, G * P),
        "b_offx": (G * P,), "b_offy": (G * P,),
        "b_offxn": (G * P,), "b_offyn": (G * P,),
        "w_msk": (C, G * P), "b_msk": (G * P,),
        "w_out": (C, C), "b_out": (C,),
        "i128": (C, C), "b_out_row": (1, C), "ones_n": (1, CHUNK),
        "eg": (G * P, G), "egt": (G, G * P),
        "m1": (G * P, 9, R1), "m2": (G * P, 9, R2),
        "ea1": (R1, NH1, C), "ea2": (R2, NH2, C),
        "ones_row": (1, C), "invc_col": (C, 1),
    }
    for name, shape in specs.items():
        dt = BF16 if name in _BF16_INPUTS else F32
        io[name] = nc.dram_tensor(name, list(shape), dt,
                                  kind="ExternalInput").ap()
    io["out"] = nc.dram_tensor("out", [POS, C], F32,
                               kind="ExternalOutput").ap()
    with tile.TileContext(nc) as tc:
        _dcn_tile(tc, io)
    _split_surplus_waits(nc)
    _CACHED["nc"] = nc
    return nc


def make_in_maps(inputs):
    consts = _host_constants(inputs)
    x = np.asarray(inputs["x"], np.float32).reshape(N, POS, C).astype(NPBF)
    base = {
        "w_in": np.asarray(inputs["w_in"], np.float32),
        "b_in": np.asarray(inputs["b_in"], np.float32),
        "dw_b": np.asarray(inputs["dw_b"], np.float32),
        "ln_g": np.asarray(inputs["ln_g"], np.float32),
        "ln_b": np.asarray(inputs["ln_b"], np.float32),
        "w_msk": np.asarray(inputs["w_msk"], np.float32),
        "b_msk": np.asarray(inputs["b_msk"], np.float32),
        "w_out": np.asarray(inputs["w_out"], np.float32),
        "b_out": np.asarray(inputs["b_out"], np.float32),
        **consts,
    }
    base = {k: (v.astype(NPBF) if k in _BF16_INPUTS else v)
            for k, v in base.items()}
    return [{**base, "x": np.ascontiguousarray(x[i])} for i in range(N)]


def kernel(**inputs):
    nc = _build_bass()
    in_maps = make_in_maps(inputs)
    from concourse.bass_utils import run_bass_kernel_spmd
    res = run_bass_kernel_spmd(nc, in_maps, list(range(N)))
    out = np.stack([res.results[i]["out"] for i in range(N)])
    return out.reshape(N, H, W, C).astype(np.float32)



# revision 6
# speedup vs baseline: 1.0118x; 1.0118x over previous
"""DCNv3 (deformable conv v3) forward as a Bass/Tile kernel for Trainium2.

Contract: kernel(**inputs) takes the FULL inputs of reference.setup_inputs()
and returns the FULL (8, 64, 64, 128) output. The batch dim (8) is
data-parallel across 8 NeuronCores; each core runs an identical single-image
program (no collectives).

Algorithm (validated vs the jax reference in numpy, rel err ~4e-6):
  x_proj = x @ w_in + b_in
  x1     = gelu(LN(dwconv3x3(x) + dw_b) * ln_g + ln_b)
  offs   = x1 @ w_off + b_off        (per group g, point p: (ox, oy), |o|<1)
  e      = exp(x1 @ w_msk + b_msk);  m = e / sum_p e
  Bilinear sampling of point p at (h+1+ky+oy, w+1+kx+ox) decomposes into
  per-axis 3-tap tents  t[-1]=relu(-o), t[0]=1-|o|, t[1]=relu(o), so the
  mask-weighted sample sum collapses to a 5x5 shift window:
     out[pos, (g,c)] = sum_{sy,sx in [-2,2]} A[(g,sy,sx), pos] *
                       xproj_pad2[pos + (sy,sx), (g,c)]
  The 9 (dy,dx) tent-product terms are re-expressed in the 9-product basis
     {m, m*tymn, m*typ, m*txmn, m*txp, m*tymn*txmn, m*tymn*txp,
      m*typ*txmn, m*typ*txp},   tmn=min(o,0), tp=max(o,0),
  whose (constant) scatter matrices fold the basis-change coefficients, so
  the on-chip work is 4 one-scalar tensor_scalar tents (4x DVE mode) and 8
  elementwise products instead of 6 slow-path tent ops and 12 products.
  xproj is padded by 2 (inner ring = conv pad inside the sampling grid,
  outer ring = zeros = grid_sample zero padding), making all window reads
  in-bounds with no boundary special cases.
  final  = out @ w_out + b_out

Layout: channel-major [C on partitions, positions on the free axis], so all
channel contractions are natural matmuls. Matmul operands are bf16 (full PE
rate + FWL weight loads); all accumulation (PSUM) is fp32. x^T arrives via
the DMA xbar transpose (no PE/ACT cost). The output projection is fused
into the apply loop: final = sum_s (A_s (*) img_s) @ w_out accumulates
across the 25 shifts in PSUM; per-shift A-broadcast tiles are consumed
three ways to balance engines: evicted to SBUF by ACT, evicted by the
(otherwise idle) Pool engine, or multiplied straight out of PSUM by DVE.
A-build and apply phases share the schedule (two concurrently-open PSUM
pools) so their dependency bubbles overlap.
"""

from contextlib import ExitStack

import ml_dtypes
import numpy as np

import concourse.bass as bass
import concourse.mybir as mybir
import concourse.tile as tile
from concourse._compat import with_exitstack

N, H, W, C, G, K = 8, 64, 64, 128, 8, 3
GC = C // G            # 16
P = K * K              # 9
POS = H * W            # 4096
HP, WP = H + 2, W + 2            # dwconv pad-1 grid (66)
HP2, WP2 = H + 4, W + 4          # sampling pad-2 grid (68)
EPS = 1e-6
NS = 25                          # 5x5 shift window
NH1, NH2 = 13, 12                # A row split: s in [0,13), [13,25)
R1, R2 = G * NH1, G * NH2        # 104, 96 partition rows of the two A halves
NB = 9                           # tent-product basis size
F32 = mybir.dt.float32
BF16 = mybir.dt.bfloat16
NPBF = ml_dtypes.bfloat16

CHUNK = 512                      # free-dim chunk for the build phase
NCH = POS // CHUNK               # 8

AF = mybir.ActivationFunctionType
OP = mybir.AluOpType

# Per-shift handling of the A-broadcast PSUM tile in the apply loop (Pool
# cannot touch PSUM, so every evict is on ACT):
#   'A' = ACT evicts to bf16 SBUF, DVE multiplies at 2x
#   'M' = ACT evicts to bf16 SBUF, Pool multiplies (slow but otherwise idle)
#   'D' = DVE multiplies straight out of PSUM (1x, but no evict at all)
APPLY_MODE = ['A', 'D', 'M', 'D', 'A',
              'D', 'A', 'M', 'D', 'A',
              'D', 'M', 'A', 'D', 'A',
              'D', 'M', 'A', 'D', 'A',
              'D', 'M', 'A', 'D', 'D']

# basis-change coefficients: d-term (dy_idx, dx_idx) -> {basis index: coeff}
# with stored tents tmn=min(o,0), tp=max(o,0) and actual taps
# t[-1]=-tmn, t[0]=1+tmn-tp, t[1]=tp.
_COEFF = {
    (0, 0): {5: 1.0},
    (0, 1): {1: -1.0, 5: -1.0, 6: 1.0},
    (0, 2): {6: -1.0},
    (1, 0): {3: -1.0, 5: -1.0, 7: 1.0},
    (1, 1): {0: 1.0, 1: 1.0, 2: -1.0, 3: 1.0, 4: -1.0,
             5: 1.0, 6: -1.0, 7: -1.0, 8: 1.0},
    (1, 2): {4: 1.0, 6: 1.0, 8: -1.0},
    (2, 0): {7: -1.0},
    (2, 1): {2: 1.0, 7: 1.0, 8: -1.0},
    (2, 2): {8: 1.0},
}


# --------------------------------------------------------------------------
# host-side constant matrices
# --------------------------------------------------------------------------

def _host_constants(inputs):
    dw_w = np.asarray(inputs["dw_w"], np.float32)        # (3,3,1,C) [ky,kx]
    w_off = np.asarray(inputs["w_off"], np.float32)      # (C, G*P*2)
    b_off = np.asarray(inputs["b_off"], np.float32)      # (G*P*2,)

    # depthwise weights as 9 diagonal matrices, c-major: [c_row, s, c_col]
    dwdiag = np.zeros((C, P, C), np.float32)
    for s in range(P):
        ky, kx = s // 3, s % 3
        dwdiag[np.arange(C), s, np.arange(C)] = dw_w[ky, kx, 0]

    w_offx = np.ascontiguousarray(w_off[:, 0::2])        # (C, 72)
    w_offy = np.ascontiguousarray(w_off[:, 1::2])
    b_offx = np.ascontiguousarray(b_off[0::2])           # (72,)
    b_offy = np.ascontiguousarray(b_off[1::2])

    # group-sum / group-broadcast 0/1 matrices for the mask softmax
    eg = np.zeros((G * P, G), np.float32)                # lhsT: (72 gp, 8 g)
    egt = np.zeros((G, G * P), np.float32)               # lhsT: (8 g, 72 gp)
    for g in range(G):
        eg[g * P:(g + 1) * P, g] = 1.0
        egt[g, g * P:(g + 1) * P] = 1.0

    # A-scatter matrices over the 9-product basis: basis term b of point
    # (g,p) lands in A row (g, s), s = (p%3 + dy)*5 + (p//3 + dx) (x-major
    # p!), weighted by the basis-change coefficient of d-term (dy,dx).
    m1 = np.zeros((G * P, NB, R1), np.float32)
    m2 = np.zeros((G * P, NB, R2), np.float32)
    for (dy, dx), cs in _COEFF.items():
        for g in range(G):
            for p in range(P):
                s = (p % 3 + dy) * 5 + (p // 3 + dx)
                for b, coef in cs.items():
                    if s < NH1:
                        m1[g * P + p, b, g * NH1 + s] += coef
                    else:
                        m2[g * P + p, b, g * NH2 + (s - NH1)] += coef

    # A-broadcast matrices: A row (g, s) -> output row (g*GC + c)
    ea1 = np.zeros((R1, NH1, C), np.float32)
    ea2 = np.zeros((R2, NH2, C), np.float32)
    for g in range(G):
        for sl in range(NH1):
            ea1[g * NH1 + sl, sl, g * GC:(g + 1) * GC] = 1.0
        for sl in range(NH2):
            ea2[g * NH2 + sl, sl, g * GC:(g + 1) * GC] = 1.0

    ones_row = np.ones((1, C), np.float32)
    invc_col = np.full((C, 1), 1.0 / C, np.float32)
    i128 = np.eye(C, dtype=np.float32)

    return {
        "dwdiag": dwdiag, "w_offx": w_offx, "w_offy": w_offy,
        "b_offx": b_offx, "b_offy": b_offy,
        "eg": eg, "egt": egt,
        "m1": m1, "m2": m2, "ea1": ea1, "ea2": ea2,
        "ones_row": ones_row, "invc_col": invc_col, "i128": i128,
    }


# names of DRAM inputs delivered as bf16 (matmul operands; x additionally
# rides the DMA xbar transpose, which needs a 2-byte dtype).
_BF16_INPUTS = {
    "x", "w_in", "dwdiag", "w_offx", "w_offy", "w_msk", "w_out",
    "eg", "egt", "m1", "m2", "ea1", "ea2", "ones_row", "invc_col",
    "i128",
}


# --------------------------------------------------------------------------
# the per-core Tile program
# --------------------------------------------------------------------------

@with_exitstack
def _dcn_tile(ctx: ExitStack, tc: tile.TileContext, io: dict):
    nc = tc.nc
    ctx.enter_context(nc.allow_low_precision(
        reason="bf16 matmul operands; accumulation stays fp32 in PSUM and "
               "in the fp32 sampling accumulator"))

    persist = ctx.enter_context(tc.tile_pool(name="persist", bufs=1))
    temps = ctx.enter_context(tc.tile_pool(name="temps", bufs=3))
    tents = ctx.enter_context(tc.tile_pool(name="tents", bufs=2))

    # ---- load weights / constants -------------------------------------
    def load(name, shape, col=False):
        dt = BF16 if name in _BF16_INPUTS else F32
        t = persist.tile(shape, dt, tag=f"w_{name}")
        src = io[name]
        if col:  # DRAM vector (n,) -> SBUF [n, 1]
            src = bass.AP(tensor=src.tensor, offset=src.offset,
                          ap=[[1, shape[0]], [1, 1]])
        nc.sync.dma_start(out=t, in_=src)
        return t

    w_in = load("w_in", [C, C])
    w_out = load("w_out", [C, C])
    w_offx = load("w_offx", [C, G * P])
    w_offy = load("w_offy", [C, G * P])
    w_msk = load("w_msk", [C, G * P])
    dwdiag = load("dwdiag", [C, P, C])          # [c_row, s, c_col]
    eg = load("eg", [G * P, G])
    egt = load("egt", [G, G * P])
    m1 = load("m1", [G * P, NB, R1])
    m2 = load("m2", [G * P, NB, R2])
    ea1 = load("ea1", [R1, NH1, C])
    ea2 = load("ea2", [R2, NH2, C])
    ones_row = load("ones_row", [1, C])
    invc_col = load("invc_col", [C, 1])
    i128 = load("i128", [C, C])
    b_in = load("b_in", [C, 1], col=True)
    dw_b = load("dw_b", [C, 1], col=True)
    ln_g = load("ln_g", [C, 1], col=True)
    ln_b = load("ln_b", [C, 1], col=True)
    b_out_c = load("b_out", [C, 1], col=True)
    b_offx = load("b_offx", [G * P, 1], col=True)
    b_offy = load("b_offy", [G * P, 1], col=True)
    b_msk = load("b_msk", [G * P, 1], col=True)

    eps1 = persist.tile([1, 1], F32)
    nc.vector.memset(eps1, EPS)

    # ---- persistent activations ---------------------------------------
    xt_pad = persist.tile([C, HP, WP], BF16)     # x^T, conv-padded (66x66)
    xpj_pad = persist.tile([C, HP2, WP2], BF16)  # x_proj^T, pad-2 (68x68)
    a1 = [persist.tile([R1, CHUNK], BF16, tag=f"a1_{i}", name=f"a1_{i}")
          for i in range(NCH)]
    a2 = [persist.tile([R2, CHUNK], BF16, tag=f"a2_{i}", name=f"a2_{i}")
          for i in range(NCH)]

    # only the pad rings need zeroing; the interiors are fully overwritten
    nc.vector.memset(xt_pad[:, 0:1, :], 0.0)
    nc.vector.memset(xt_pad[:, HP - 1:HP, :], 0.0)
    nc.vector.memset(xt_pad[:, 1:HP - 1, 0:1], 0.0)
    nc.vector.memset(xt_pad[:, 1:HP - 1, WP - 1:WP], 0.0)
    nc.vector.memset(xpj_pad[:, 0:2, :], 0.0)
    nc.vector.memset(xpj_pad[:, HP2 - 2:HP2, :], 0.0)
    nc.vector.memset(xpj_pad[:, 2:HP2 - 2, 0:2], 0.0)
    nc.vector.memset(xpj_pad[:, 2:HP2 - 2, WP2 - 2:WP2], 0.0)

    # ---- stage 1: x^T via the DMA xbar transpose ----------------------
    # The interpreter's transpose semantics only match AP-linear order for
    # 2D outputs, so land in a contiguous tile and restride with one DVE
    # copy (4x mode) into the padded grid.
    xt_flat = persist.tile([C, POS], BF16)
    nc.sync.dma_start(out=xt_flat, in_=io["x"], transpose=True)
    nc.vector.tensor_copy(out=xt_pad[:, 1:1 + H, 1:1 + W], in_=xt_flat)

    # ---- stage 2: x_proj into the pad-2 grid --------------------------
    with tc.tile_pool(name="ps_s12", bufs=2, space="PSUM") as psum:
        for ch in range(NCH):        # chunk = 8 h-rows
            h0 = ch * 8
            rhs = xt_pad[:, 1 + h0:1 + h0 + 8, 1:1 + W]
            ps = psum.tile([C, CHUNK], F32, tag="ps_proj")
            nc.tensor.matmul(ps, w_in, rhs, start=True, stop=True)
            nc.scalar.activation(
                out=xpj_pad[:, 2 + h0:2 + h0 + 8, 2:2 + W],
                in_=ps.rearrange("c (a b) -> c a b", b=W),
                func=AF.Identity, bias=b_in, scale=1.0)

    # ---- stages 3+4, interleaved ---------------------------------------
    # Two PSUM pools, both open across the whole fused phase so the
    # scheduler can interleave A-building and applying freely.
    with tc.tile_pool(name="ps3", bufs=1, space="PSUM") as psum, \
            tc.tile_pool(name="ps4", bufs=1, space="PSUM") as psum4:

        def build_chunk(ch):
            h0 = ch * 8
            # depthwise conv via 9 diagonal matmuls
            psA = psum.tile([C, CHUNK], F32, tag="psmm", bufs=1, name="psA")
            for s in range(P):
                ky, kx = s // 3, s % 3
                rhs = xt_pad[:, h0 + ky:h0 + ky + 8, kx:kx + W]
                nc.tensor.matmul(psA, dwdiag[:, s, :], rhs,
                                 start=(s == 0), stop=(s == P - 1))
            x1c = temps.tile([C, CHUNK], BF16, tag="x1c")
            nc.scalar.activation(out=x1c, in_=psA, func=AF.Identity,
                                 bias=dw_b, scale=1.0)

            # LayerNorm over channels (partition dim) via 1/C col matmuls
            x1sq = temps.tile([C, CHUNK], BF16, tag="scr")
            nc.vector.tensor_mul(out=x1sq, in0=x1c, in1=x1c)
            psm = psum.tile([1, CHUNK], F32, tag="psst", bufs=1, name="psm")
            nc.tensor.matmul(psm, invc_col, x1c, start=True, stop=True)
            mean_r = temps.tile([1, CHUNK], BF16, tag="mean_r")
            nc.scalar.copy(out=mean_r, in_=psm)
            psq = psum.tile([1, CHUNK], F32, tag="psst", bufs=1, name="psq")
            nc.tensor.matmul(psq, invc_col, x1sq, start=True, stop=True)
            tmp_r = temps.tile([1, CHUNK], F32, tag="tmp_r")
            nc.scalar.activation(out=tmp_r, in_=mean_r, func=AF.Square)
            nc.vector.tensor_sub(out=tmp_r, in0=psq, in1=tmp_r)   # var
            nc.scalar.activation(out=tmp_r, in_=tmp_r, func=AF.Sqrt,
                                 bias=eps1, scale=1.0)            # std
            rstd_r = temps.tile([1, CHUNK], BF16, tag="rstd_r")
            nc.vector.reciprocal(out=rstd_r, in_=tmp_r)           # rstd
            mrs_r = temps.tile([1, CHUNK], BF16, tag="mrs_r")
            nc.vector.tensor_mul(out=mrs_r, in0=mean_r, in1=rstd_r)
            psR = psum.tile([C, CHUNK], F32, tag="psmm", bufs=1, name="psR")
            nc.tensor.matmul(psR, ones_row, rstd_r, start=True, stop=True)
            zc = temps.tile([C, CHUNK], F32, tag="scr2")
            nc.vector.tensor_mul(out=zc, in0=x1c, in1=psR)
            psM = psum.tile([C, CHUNK], F32, tag="psmm", bufs=1, name="psM")
            nc.tensor.matmul(psM, ones_row, mrs_r, start=True, stop=True)
            nc.vector.tensor_sub(out=zc, in0=zc, in1=psM)
            x1gc = temps.tile([C, CHUNK], BF16, tag="x1gc")
            nc.scalar.activation(out=x1gc, in_=zc, func=AF.Gelu,
                                 bias=ln_b, scale=ln_g)

            # offset projections; tents as one-scalar tensor_scalar ops on
            # bf16 SBUF evictions (4x DVE mode)
            psX = psum.tile([G * P, CHUNK], F32, tag="psoff", bufs=1,
                            name="psX")
            nc.tensor.matmul(psX, w_offx, x1gc, start=True, stop=True)
            oxs = tents.tile([G * P, CHUNK], BF16, tag="oxs")
            nc.scalar.activation(out=oxs, in_=psX, func=AF.Identity,
                                 bias=b_offx, scale=1.0)
            txmn = tents.tile([G * P, CHUNK], BF16, tag="txmn")
            txp = tents.tile([G * P, CHUNK], BF16, tag="txp")
            nc.vector.tensor_scalar(out=txmn, in0=oxs, scalar1=0.0,
                                    scalar2=None, op0=OP.min)
            nc.vector.tensor_scalar(out=txp, in0=oxs, scalar1=0.0,
                                    scalar2=None, op0=OP.max)
            psY = psum.tile([G * P, CHUNK], F32, tag="psoff", bufs=1,
                            name="psY")
            nc.tensor.matmul(psY, w_offy, x1gc, start=True, stop=True)
            oys = tents.tile([G * P, CHUNK], BF16, tag="oys")
            nc.scalar.activation(out=oys, in_=psY, func=AF.Identity,
                                 bias=b_offy, scale=1.0)
            tymn = tents.tile([G * P, CHUNK], BF16, tag="tymn")
            typ = tents.tile([G * P, CHUNK], BF16, tag="typ")
            nc.vector.tensor_scalar(out=tymn, in0=oys, scalar1=0.0,
                                    scalar2=None, op0=OP.min)
            nc.vector.tensor_scalar(out=typ, in0=oys, scalar1=0.0,
                                    scalar2=None, op0=OP.max)

            # normalized mask: e_n = exp(l + b) / group sum
            psE = psum.tile([G * P, CHUNK], F32, tag="psoff", bufs=1,
                            name="psE")
            nc.tensor.matmul(psE, w_msk, x1gc, start=True, stop=True)
            ec = temps.tile([G * P, CHUNK], BF16, tag="ec")
            nc.scalar.activation(out=ec, in_=psE, func=AF.Exp,
                                 bias=b_msk, scale=1.0)
            psS = psum.tile([G, CHUNK], F32, tag="psoff", bufs=1,
                            name="psS")
            nc.tensor.matmul(psS, eg, ec, start=True, stop=True)
            s_r = temps.tile([G, CHUNK], BF16, tag="s_r")
            nc.vector.reciprocal(out=s_r, in_=psS)
            psB = psum.tile([G * P, CHUNK], F32, tag="psoff", bufs=1,
                            name="psB")
            nc.tensor.matmul(psB, egt, s_r, start=True, stop=True)
            nc.vector.tensor_mul(out=ec, in0=ec, in1=psB)

            # 8 basis products (all-bf16 SBUF tensor_tensor, 2x mode)
            b1 = tents.tile([G * P, CHUNK], BF16, tag="b1")
            b2 = tents.tile([G * P, CHUNK], BF16, tag="b2")
            b3 = tents.tile([G * P, CHUNK], BF16, tag="b3")
            b4 = tents.tile([G * P, CHUNK], BF16, tag="b4")
            b5 = tents.tile([G * P, CHUNK], BF16, tag="b5")
            b6 = tents.tile([G * P, CHUNK], BF16, tag="b6")
            b7 = tents.tile([G * P, CHUNK], BF16, tag="b7")
            b8 = tents.tile([G * P, CHUNK], BF16, tag="b8")
            nc.vector.tensor_mul(out=b1, in0=ec, in1=tymn)
            nc.vector.tensor_mul(out=b2, in0=ec, in1=typ)
            nc.vector.tensor_mul(out=b3, in0=ec, in1=txmn)
            nc.vector.tensor_mul(out=b4, in0=ec, in1=txp)
            nc.gpsimd.tensor_mul(out=b5, in0=b1, in1=txmn)
            nc.vector.tensor_mul(out=b6, in0=b1, in1=txp)
            nc.gpsimd.tensor_mul(out=b7, in0=b2, in1=txmn)
            nc.vector.tensor_mul(out=b8, in0=b2, in1=txp)
            basis = [ec, b1, b2, b3, b4, b5, b6, b7, b8]

            psA1 = psum.tile([R1, CHUNK], F32, tag="psa1", bufs=1,
                             name="psA1")
            psA2 = psum.tile([R2, CHUNK], F32, tag="psa2", bufs=1,
                             name="psA2")
            for b in range(NB):
                nc.tensor.matmul(psA1, m1[:, b, :], basis[b],
                                 start=(b == 0), stop=(b == NB - 1))
                nc.tensor.matmul(psA2, m2[:, b, :], basis[b],
                                 start=(b == 0), stop=(b == NB - 1))
            nc.scalar.copy(out=a1[ch], in_=psA1)
            nc.scalar.copy(out=a2[ch], in_=psA2)

        def apply_chunk(ch):
            h0 = ch * 8            # 8 h-rows per 512-pos chunk
            outp = psum4.tile([C, CHUNK], F32, tag="psoacc", bufs=1,
                              name="outp")
            for s in range(NS):
                sy, sx = s // 5 - 2, s % 5 - 2
                if s < NH1:
                    lhsT, arows = ea1[:, s, :], a1[ch]
                else:
                    lhsT, arows = ea2[:, s - NH1, :], a2[ch]
                psBc = psum4.tile([C, CHUNK], F32, tag="psab", bufs=2,
                                  name="psBc")
                nc.tensor.matmul(psBc, lhsT, arows, start=True, stop=True)
                row = 2 + sy + h0
                img = xpj_pad[:, row:row + 8, 2 + sx:2 + sx + W]
                t = temps.tile([C, CHUNK], BF16, tag="t_app")
                mode = APPLY_MODE[s]
                if mode == 'D':      # multiply straight out of PSUM
                    nc.vector.tensor_mul(out=t, in0=psBc, in1=img)
                else:
                    ab = temps.tile([C, CHUNK], BF16,
                                    tag="ab_a" if mode == 'A' else "ab_p")
                    nc.scalar.copy(out=ab, in_=psBc)
                    if mode == 'A':
                        nc.vector.tensor_mul(out=t, in0=ab, in1=img)
                    else:
                        nc.gpsimd.tensor_mul(out=t, in0=ab, in1=img)
                nc.tensor.matmul(outp, w_out, t, start=(s == 0),
                                 stop=(s == NS - 1))

            # evict (+bias), transpose to pos-major via identity matmuls
            fsb = temps.tile([C, CHUNK], BF16, tag="fsb")
            nc.scalar.activation(out=fsb, in_=outp, func=AF.Identity,
                                 bias=b_out_c, scale=1.0)
            pso = psum4.tile([C, 4, C], F32, tag="psab", bufs=2, name="pso")
            for q in range(4):
                nc.tensor.matmul(pso[:, q, :], fsb[:, q * C:(q + 1) * C],
                                 i128, start=True, stop=True)
            osb = temps.tile([C, 4, C], F32, tag="osb")
            nc.scalar.copy(out=osb, in_=pso)
            pos0 = ch * CHUNK
            nc.sync.dma_start(
                out=bass.AP(tensor=io["out"].tensor,
                            offset=io["out"].offset + pos0 * C,
                            ap=[[C, C], [C * C, 4], [1, C]]),
                in_=osb)

        for ch in range(NCH):
            build_chunk(ch)
            if ch > 0:
                apply_chunk(ch - 1)
        apply_chunk(NCH - 1)


# --------------------------------------------------------------------------
# bass module build + public entry point
# --------------------------------------------------------------------------

# Hardware TPB instructions carry exactly ONE sync-wait slot (the
# NEURON_ISA_TPB_EVENTS struct).  Tile can emit several waits on one BIR
# instruction; walrus splits matmult waits across the LDWEIGHTS/MATMULT
# pair, but single-struct ops (Activation, ...) fail codegen with "Too many
# sync wait commands".  Move surplus waits onto standalone same-engine
# EventSemaphore instructions inserted immediately before the offender.
def _wait_cap(ins):
    t = type(ins).__name__
    if t == "InstEventSemaphore":
        return None
    return 1


def _split_surplus_waits(nc):
    import bass_rust
    n = 0
    for bb in nc.m.functions[0].blocks:
        out = []
        for ins in bb.instructions:
            si = getattr(ins, "sync_info", None)
            cap = _wait_cap(ins)
            if si is not None and cap is not None and len(si.on_wait) > cap:
                waits = list(si.on_wait)
                for i, w in enumerate(waits[:-cap]):
                    ev = mybir.InstEventSemaphore(
                        name=f"{ins.name}_xw{i}", ins=[], outs=[])
                    ev.engine = ins.engine
                    ev.sync_info = bass_rust.SyncInfo(on_wait=[w],
                                                     on_update=[])
                    nc.register_instruction(ev)
                    out.append(ev)
                    n += 1
                ins.sync_info = bass_rust.SyncInfo(
                    on_wait=waits[-cap:], on_update=list(si.on_update))
            out.append(ins)
        bb.instructions = out
    return n


_CACHED = {}


def _build_bass():
    if "nc" in _CACHED:
        return _CACHED["nc"]
    nc = bass.Bass()
    io = {}
    specs = {
        "x": (POS, C), "w_in": (C, C), "b_in": (C,), "dwdiag": (C, P, C),
        "dw_b": (C,), "ln_g": (C,), "ln_b": (C,),
        "w_offx": (C, G * P), "w_offy": (C, G * P),
        "b_offx": (G * P,), "b_offy": (G * P,),
        "w_msk": (C, G * P), "b_msk": (G * P,),
        "w_out": (C, C), "b_out": (C,),
        "i128": (C, C),
        "eg": (G * P, G), "egt": (G, G * P),
        "m1": (G * P, NB, R1), "m2": (G * P, NB, R2),
        "ea1": (R1, NH1, C), "ea2": (R2, NH2, C),
        "ones_row": (1, C), "invc_col": (C, 1),
    }
    for name, shape in specs.items():
        dt = BF16 if name in _BF16_INPUTS else F32
        io[name] = nc.dram_tensor(name, list(shape), dt,
                                  kind="ExternalInput").ap()
    io["out"] = nc.dram_tensor("out", [POS, C], F32,
                               kind="ExternalOutput").ap()
    with tile.TileContext(nc) as tc:
        _dcn_tile(tc, io)
    _split_surplus_waits(nc)
    _CACHED["nc"] = nc
    return nc


def make_in_maps(inputs):
    consts = _host_constants(inputs)
    x = np.asarray(inputs["x"], np.float32).reshape(N, POS, C).astype(NPBF)
    base = {
        "w_in": np.asarray(inputs["w_in"], np.float32),
        "b_in": np.asarray(inputs["b_in"], np.float32),
        "dw_b": np.asarray(inputs["dw_b"], np.float32),
        "ln_g": np.asarray(inputs["ln_g"], np.float32),
        "ln_b": np.asarray(inputs["ln_b"], np.float32),
        "w_msk": np.asarray(inputs["w_msk"], np.float32),
        "b_msk": np.asarray(inputs["b_msk"], np.float32),
        "w_out": np.asarray(inputs["w_out"], np.float32),
        "b_out": np.asarray(inputs["b_out"], np.float32),
        **consts,
    }
    base = {k: (v.astype(NPBF) if k in _BF16_INPUTS else v)
            for k, v in base.items()}
    return [{**base, "x": np.ascontiguousarray(x[i])} for i in range(N)]


def kernel(**inputs):
    nc = _build_bass()
    in_maps = make_in_maps(inputs)
    from concourse.bass_utils import run_bass_kernel_spmd
    res = run_bass_kernel_spmd(nc, in_maps, list(range(N)))
    out = np.stack([res.results[i]["out"] for i in range(N)])
    return out.reshape(N, H, W, C).astype(np.float32)


# revision 8
# speedup vs baseline: 1.0495x; 1.0373x over previous
"""DCNv3 (deformable conv v3) forward as a Bass/Tile kernel for Trainium2.

Contract: kernel(**inputs) takes the FULL inputs of reference.setup_inputs()
and returns the FULL (8, 64, 64, 128) output. The batch dim (8) is
data-parallel across 8 NeuronCores; each core runs an identical single-image
program (no collectives).

Algorithm (validated vs the jax reference in numpy, rel err ~4e-6):
  x_proj = x @ w_in + b_in
  x1     = gelu(LN(dwconv3x3(x) + dw_b) * ln_g + ln_b)
  offs   = x1 @ w_off + b_off        (per group g, point p: (ox, oy), |o|<1)
  e      = exp(x1 @ w_msk + b_msk);  m = e / sum_p e
  Bilinear sampling of point p at (h+1+ky+oy, w+1+kx+ox) decomposes into
  per-axis 3-tap tents  t[-1]=relu(-o), t[0]=1-|o|, t[1]=relu(o), so the
  mask-weighted sample sum collapses to a 5x5 shift window:
     out[pos, (g,c)] = sum_{sy,sx in [-2,2]} A[(g,sy,sx), pos] *
                       xproj_pad2[pos + (sy,sx), (g,c)]
  The 9 (dy,dx) tent-product terms are re-expressed in the 9-product basis
     {m, m*tymn, m*typ, m*txmn, m*txp, m*tymn*txmn, m*tymn*txp,
      m*typ*txmn, m*typ*txp},   tmn=min(o,0), tp=max(o,0),
  whose (constant) scatter matrices fold the basis-change coefficients, so
  the on-chip work is 4 one-scalar tensor_scalar tents (4x DVE mode) and 8
  elementwise products instead of 6 slow-path tent ops and 12 products.
  xproj is padded by 2 (inner ring = conv pad inside the sampling grid,
  outer ring = zeros = grid_sample zero padding), making all window reads
  in-bounds with no boundary special cases.
  final  = out @ w_out + b_out

Layout: channel-major [C on partitions, positions on the free axis], so all
channel contractions are natural matmuls. Matmul operands are bf16 (full PE
rate + FWL weight loads); all accumulation (PSUM) is fp32. x^T arrives via
the DMA xbar transpose (no PE/ACT cost). The output projection is fused
into the apply loop: final = sum_s (A_s (*) img_s) @ w_out accumulates
across the 25 shifts in PSUM; per-shift A-broadcast tiles are consumed
three ways to balance engines: evicted to SBUF by ACT, evicted by the
(otherwise idle) Pool engine, or multiplied straight out of PSUM by DVE.
A-build and apply phases share the schedule (two concurrently-open PSUM
pools) so their dependency bubbles overlap.
"""

from contextlib import ExitStack

import ml_dtypes
import numpy as np

import concourse.bass as bass
import concourse.mybir as mybir
import concourse.tile as tile
from concourse._compat import with_exitstack

N, H, W, C, G, K = 8, 64, 64, 128, 8, 3
GC = C // G            # 16
P = K * K              # 9
POS = H * W            # 4096
HP, WP = H + 2, W + 2            # dwconv pad-1 grid (66)
HP2, WP2 = H + 4, W + 4          # sampling pad-2 grid (68)
EPS = 1e-6
NS = 25                          # 5x5 shift window
NH1, NH2 = 13, 12                # A row split: s in [0,13), [13,25)
R1, R2 = G * NH1, G * NH2        # 104, 96 partition rows of the two A halves
NB = 9                           # tent-product basis size
F32 = mybir.dt.float32
BF16 = mybir.dt.bfloat16
NPBF = ml_dtypes.bfloat16

CHUNK = 512                      # free-dim chunk for the build phase
NCH = POS // CHUNK               # 8

AF = mybir.ActivationFunctionType
OP = mybir.AluOpType

# Per-shift handling of the A-broadcast PSUM tile in the apply loop (Pool
# cannot touch PSUM, so every evict is on ACT):
#   'A' = ACT evicts to bf16 SBUF, DVE multiplies at 2x
#   'M' = ACT evicts to bf16 SBUF, Pool multiplies (slow but otherwise idle)
#   'D' = DVE multiplies straight out of PSUM (1x, but no evict at all)
APPLY_MODE = ['A', 'D', 'M', 'D', 'A',
              'D', 'A', 'M', 'D', 'A',
              'D', 'M', 'A', 'D', 'A',
              'D', 'M', 'A', 'D', 'A',
              'D', 'M', 'A', 'D', 'D']

# basis-change coefficients: d-term (dy_idx, dx_idx) -> {basis index: coeff}
# with stored tents tmn=min(o,0), tp=max(o,0) and actual taps
# t[-1]=-tmn, t[0]=1+tmn-tp, t[1]=tp.
_COEFF = {
    (0, 0): {5: 1.0},
    (0, 1): {1: -1.0, 5: -1.0, 6: 1.0},
    (0, 2): {6: -1.0},
    (1, 0): {3: -1.0, 5: -1.0, 7: 1.0},
    (1, 1): {0: 1.0, 1: 1.0, 2: -1.0, 3: 1.0, 4: -1.0,
             5: 1.0, 6: -1.0, 7: -1.0, 8: 1.0},
    (1, 2): {4: 1.0, 6: 1.0, 8: -1.0},
    (2, 0): {7: -1.0},
    (2, 1): {2: 1.0, 7: 1.0, 8: -1.0},
    (2, 2): {8: 1.0},
}


# --------------------------------------------------------------------------
# host-side constant matrices
# --------------------------------------------------------------------------

def _host_constants(inputs):
    dw_w = np.asarray(inputs["dw_w"], np.float32)        # (3,3,1,C) [ky,kx]
    w_off = np.asarray(inputs["w_off"], np.float32)      # (C, G*P*2)
    b_off = np.asarray(inputs["b_off"], np.float32)      # (G*P*2,)

    # depthwise weights as 9 diagonal matrices, c-major: [c_row, s, c_col]
    dwdiag = np.zeros((C, P, C), np.float32)
    for s in range(P):
        ky, kx = s // 3, s % 3
        dwdiag[np.arange(C), s, np.arange(C)] = dw_w[ky, kx, 0]

    w_offx = np.ascontiguousarray(w_off[:, 0::2])        # (C, 72)
    w_offy = np.ascontiguousarray(w_off[:, 1::2])
    b_offx = np.ascontiguousarray(b_off[0::2])           # (72,)
    b_offy = np.ascontiguousarray(b_off[1::2])

    # group-sum / group-broadcast 0/1 matrices for the mask softmax
    eg = np.zeros((G * P, G), np.float32)                # lhsT: (72 gp, 8 g)
    egt = np.zeros((G, G * P), np.float32)               # lhsT: (8 g, 72 gp)
    for g in range(G):
        eg[g * P:(g + 1) * P, g] = 1.0
        egt[g, g * P:(g + 1) * P] = 1.0

    # A-scatter matrices over the 9-product basis: basis term b of point
    # (g,p) lands in A row (g, s), s = (p%3 + dy)*5 + (p//3 + dx) (x-major
    # p!), weighted by the basis-change coefficient of d-term (dy,dx).
    m1 = np.zeros((G * P, NB, R1), np.float32)
    m2 = np.zeros((G * P, NB, R2), np.float32)
    for (dy, dx), cs in _COEFF.items():
        for g in range(G):
            for p in range(P):
                s = (p % 3 + dy) * 5 + (p // 3 + dx)
                for b, coef in cs.items():
                    if s < NH1:
                        m1[g * P + p, b, g * NH1 + s] += coef
                    else:
                        m2[g * P + p, b, g * NH2 + (s - NH1)] += coef

    # A-broadcast matrices: A row (g, s) -> output row (g*GC + c)
    ea1 = np.zeros((R1, NH1, C), np.float32)
    ea2 = np.zeros((R2, NH2, C), np.float32)
    for g in range(G):
        for sl in range(NH1):
            ea1[g * NH1 + sl, sl, g * GC:(g + 1) * GC] = 1.0
        for sl in range(NH2):
            ea2[g * NH2 + sl, sl, g * GC:(g + 1) * GC] = 1.0

    ones_row = np.ones((1, C), np.float32)
    invc_col = np.full((C, 1), 1.0 / C, np.float32)
    i128 = np.eye(C, dtype=np.float32)

    return {
        "dwdiag": dwdiag, "w_offx": w_offx, "w_offy": w_offy,
        "b_offx": b_offx, "b_offy": b_offy,
        "eg": eg, "egt": egt,
        "m1": m1, "m2": m2, "ea1": ea1, "ea2": ea2,
        "ones_row": ones_row, "invc_col": invc_col, "i128": i128,
    }


# names of DRAM inputs delivered as bf16 (matmul operands; x additionally
# rides the DMA xbar transpose, which needs a 2-byte dtype).
_BF16_INPUTS = {
    "x", "w_in", "dwdiag", "w_offx", "w_offy", "w_msk", "w_out",
    "eg", "egt", "m1", "m2", "ea1", "ea2", "ones_row", "invc_col",
    "i128",
}


# --------------------------------------------------------------------------
# the per-core Tile program
# --------------------------------------------------------------------------

@with_exitstack
def _dcn_tile(ctx: ExitStack, tc: tile.TileContext, io: dict):
    nc = tc.nc
    ctx.enter_context(nc.allow_low_precision(
        reason="bf16 matmul operands; accumulation stays fp32 in PSUM and "
               "in the fp32 sampling accumulator"))

    persist = ctx.enter_context(tc.tile_pool(name="persist", bufs=1))
    temps = ctx.enter_context(tc.tile_pool(name="temps", bufs=3))
    tents = ctx.enter_context(tc.tile_pool(name="tents", bufs=2))

    # ---- load weights / constants -------------------------------------
    def load(name, shape, col=False):
        dt = BF16 if name in _BF16_INPUTS else F32
        t = persist.tile(shape, dt, tag=f"w_{name}")
        src = io[name]
        if col:  # DRAM vector (n,) -> SBUF [n, 1]
            src = bass.AP(tensor=src.tensor, offset=src.offset,
                          ap=[[1, shape[0]], [1, 1]])
        nc.sync.dma_start(out=t, in_=src)
        return t

    w_in = load("w_in", [C, C])
    w_out = load("w_out", [C, C])
    w_offx = load("w_offx", [C, G * P])
    w_offy = load("w_offy", [C, G * P])
    w_msk = load("w_msk", [C, G * P])
    dwdiag = load("dwdiag", [C, P, C])          # [c_row, s, c_col]
    eg = load("eg", [G * P, G])
    egt = load("egt", [G, G * P])
    m1 = load("m1", [G * P, NB, R1])
    m2 = load("m2", [G * P, NB, R2])
    ea1 = load("ea1", [R1, NH1, C])
    ea2 = load("ea2", [R2, NH2, C])
    ones_row = load("ones_row", [1, C])
    invc_col = load("invc_col", [C, 1])
    i128 = load("i128", [C, C])
    b_in = load("b_in", [C, 1], col=True)
    dw_b = load("dw_b", [C, 1], col=True)
    ln_g = load("ln_g", [C, 1], col=True)
    ln_b = load("ln_b", [C, 1], col=True)
    b_out_c = load("b_out", [C, 1], col=True)
    b_offx = load("b_offx", [G * P, 1], col=True)
    b_offy = load("b_offy", [G * P, 1], col=True)
    b_msk = load("b_msk", [G * P, 1], col=True)

    eps1 = persist.tile([1, 1], F32)
    nc.vector.memset(eps1, EPS)

    # ---- persistent activations ---------------------------------------
    xt_pad = persist.tile([C, HP, WP], BF16)     # x^T, conv-padded (66x66)
    xpj_pad = persist.tile([C, HP2, WP2], BF16)  # x_proj^T, pad-2 (68x68)
    a1 = [persist.tile([R1, CHUNK], BF16, tag=f"a1_{i}", name=f"a1_{i}")
          for i in range(NCH)]
    a2 = [persist.tile([R2, CHUNK], BF16, tag=f"a2_{i}", name=f"a2_{i}")
          for i in range(NCH)]

    # only the pad rings need zeroing; the interiors are fully overwritten
    nc.vector.memset(xt_pad[:, 0:1, :], 0.0)
    nc.vector.memset(xt_pad[:, HP - 1:HP, :], 0.0)
    nc.vector.memset(xt_pad[:, 1:HP - 1, 0:1], 0.0)
    nc.vector.memset(xt_pad[:, 1:HP - 1, WP - 1:WP], 0.0)
    nc.vector.memset(xpj_pad[:, 0:2, :], 0.0)
    nc.vector.memset(xpj_pad[:, HP2 - 2:HP2, :], 0.0)
    nc.vector.memset(xpj_pad[:, 2:HP2 - 2, 0:2], 0.0)
    nc.vector.memset(xpj_pad[:, 2:HP2 - 2, WP2 - 2:WP2], 0.0)

    # ---- stage 1: x^T via the DMA xbar transpose ----------------------
    # The interpreter's transpose semantics only match AP-linear order for
    # 2D outputs, so land in a contiguous tile and restride with one DVE
    # copy (4x mode) into the padded grid.
    xt_flat = persist.tile([C, POS], BF16)
    nc.sync.dma_start(out=xt_flat, in_=io["x"], transpose=True)
    nc.vector.tensor_copy(out=xt_pad[:, 1:1 + H, 1:1 + W], in_=xt_flat)

    # ---- stage 2: x_proj into the pad-2 grid --------------------------
    with tc.tile_pool(name="ps_s12", bufs=2, space="PSUM") as psum:
        for ch in range(NCH):        # chunk = 8 h-rows
            h0 = ch * 8
            rhs = xt_pad[:, 1 + h0:1 + h0 + 8, 1:1 + W]
            ps = psum.tile([C, CHUNK], F32, tag="ps_proj")
            nc.tensor.matmul(ps, w_in, rhs, start=True, stop=True)
            nc.scalar.activation(
                out=xpj_pad[:, 2 + h0:2 + h0 + 8, 2:2 + W],
                in_=ps.rearrange("c (a b) -> c a b", b=W),
                func=AF.Identity, bias=b_in, scale=1.0)

    # ---- stages 3+4, interleaved ---------------------------------------
    # Two PSUM pools, both open across the whole fused phase so the
    # scheduler can interleave A-building and applying freely.
    with tc.tile_pool(name="ps3", bufs=1, space="PSUM") as psum, \
            tc.tile_pool(name="ps4", bufs=1, space="PSUM") as psum4:

        def build_a(ch):
            """dwconv + the [1,512] LayerNorm stats sub-chain.  Emitted
            BEFORE apply(ch-1) so the long serial LN chain percolates
            through ACT/DVE/Pool while PE grinds the apply matmuls."""
            h0 = ch * 8
            # depthwise conv via 9 diagonal matmuls
            psA = psum.tile([C, CHUNK], F32, tag="psmm", bufs=1, name="psA")
            for s in range(P):
                ky, kx = s // 3, s % 3
                rhs = xt_pad[:, h0 + ky:h0 + ky + 8, kx:kx + W]
                nc.tensor.matmul(psA, dwdiag[:, s, :], rhs,
                                 start=(s == 0), stop=(s == P - 1))
            x1c = temps.tile([C, CHUNK], BF16, tag="x1c")
            nc.scalar.activation(out=x1c, in_=psA, func=AF.Identity,
                                 bias=dw_b, scale=1.0)

            # LayerNorm stats over channels (partition dim) via 1/C cols;
            # x1sq on the Pool engine so it doesn't queue behind the
            # previous chunk's apply muls on DVE.
            x1sq = temps.tile([C, CHUNK], BF16, tag="scr")
            nc.gpsimd.tensor_mul(out=x1sq, in0=x1c, in1=x1c)
            psm = psum.tile([1, CHUNK], F32, tag="psst", bufs=1, name="psm")
            nc.tensor.matmul(psm, invc_col, x1c, start=True, stop=True)
            mean_r = temps.tile([1, CHUNK], BF16, tag="mean_r")
            nc.scalar.copy(out=mean_r, in_=psm)
            psq = psum.tile([1, CHUNK], F32, tag="psst", bufs=1, name="psq")
            nc.tensor.matmul(psq, invc_col, x1sq, start=True, stop=True)
            tmp_r = temps.tile([1, CHUNK], F32, tag="tmp_r")
            nc.scalar.activation(out=tmp_r, in_=mean_r, func=AF.Square)
            nc.vector.tensor_sub(out=tmp_r, in0=psq, in1=tmp_r)   # var
            nc.scalar.activation(out=tmp_r, in_=tmp_r, func=AF.Sqrt,
                                 bias=eps1, scale=1.0)            # std
            rstd_r = temps.tile([1, CHUNK], BF16, tag="rstd_r")
            nc.vector.reciprocal(out=rstd_r, in_=tmp_r)           # rstd
            mrs_r = temps.tile([1, CHUNK], BF16, tag="mrs_r")
            nc.vector.tensor_mul(out=mrs_r, in0=mean_r, in1=rstd_r)
            return x1c, rstd_r, mrs_r

        def build_b(ch, x1c, rstd_r, mrs_r):
            h0 = ch * 8
            psR = psum.tile([C, CHUNK], F32, tag="psmm", bufs=1, name="psR")
            nc.tensor.matmul(psR, ones_row, rstd_r, start=True, stop=True)
            zc = temps.tile([C, CHUNK], F32, tag="scr2")
            nc.vector.tensor_mul(out=zc, in0=x1c, in1=psR)
            psM = psum.tile([C, CHUNK], F32, tag="psmm", bufs=1, name="psM")
            nc.tensor.matmul(psM, ones_row, mrs_r, start=True, stop=True)
            nc.vector.tensor_sub(out=zc, in0=zc, in1=psM)
            x1gc = temps.tile([C, CHUNK], BF16, tag="x1gc")
            nc.scalar.activation(out=x1gc, in_=zc, func=AF.Gelu,
                                 bias=ln_b, scale=ln_g)

            # offset projections; tents as one-scalar tensor_scalar ops on
            # bf16 SBUF evictions (4x DVE mode)
            psX = psum.tile([G * P, CHUNK], F32, tag="psoff", bufs=1,
                            name="psX")
            nc.tensor.matmul(psX, w_offx, x1gc, start=True, stop=True)
            oxs = tents.tile([G * P, CHUNK], BF16, tag="oxs")
            nc.scalar.activation(out=oxs, in_=psX, func=AF.Identity,
                                 bias=b_offx, scale=1.0)
            txmn = tents.tile([G * P, CHUNK], BF16, tag="txmn")
            txp = tents.tile([G * P, CHUNK], BF16, tag="txp")
            nc.vector.tensor_scalar(out=txmn, in0=oxs, scalar1=0.0,
                                    scalar2=None, op0=OP.min)
            nc.vector.tensor_scalar(out=txp, in0=oxs, scalar1=0.0,
                                    scalar2=None, op0=OP.max)
            psY = psum.tile([G * P, CHUNK], F32, tag="psoff", bufs=1,
                            name="psY")
            nc.tensor.matmul(psY, w_offy, x1gc, start=True, stop=True)
            oys = tents.tile([G * P, CHUNK], BF16, tag="oys")
            nc.scalar.activation(out=oys, in_=psY, func=AF.Identity,
                                 bias=b_offy, scale=1.0)
            tymn = tents.tile([G * P, CHUNK], BF16, tag="tymn")
            typ = tents.tile([G * P, CHUNK], BF16, tag="typ")
            nc.vector.tensor_scalar(out=tymn, in0=oys, scalar1=0.0,
                                    scalar2=None, op0=OP.min)
            nc.vector.tensor_scalar(out=typ, in0=oys, scalar1=0.0,
                                    scalar2=None, op0=OP.max)

            # normalized mask: e_n = exp(l + b) / group sum
            psE = psum.tile([G * P, CHUNK], F32, tag="psoff", bufs=1,
                            name="psE")
            nc.tensor.matmul(psE, w_msk, x1gc, start=True, stop=True)
            ec = temps.tile([G * P, CHUNK], BF16, tag="ec")
            nc.scalar.activation(out=ec, in_=psE, func=AF.Exp,
                                 bias=b_msk, scale=1.0)
            psS = psum.tile([G, CHUNK], F32, tag="psoff", bufs=1,
                            name="psS")
            nc.tensor.matmul(psS, eg, ec, start=True, stop=True)
            s_r = temps.tile([G, CHUNK], BF16, tag="s_r")
            nc.vector.reciprocal(out=s_r, in_=psS)
            psB = psum.tile([G * P, CHUNK], F32, tag="psoff", bufs=1,
                            name="psB")
            nc.tensor.matmul(psB, egt, s_r, start=True, stop=True)
            nc.vector.tensor_mul(out=ec, in0=ec, in1=psB)

            # 8 basis products (all-bf16 SBUF tensor_tensor, 2x mode)
            b1 = tents.tile([G * P, CHUNK], BF16, tag="b1")
            b2 = tents.tile([G * P, CHUNK], BF16, tag="b2")
            b3 = tents.tile([G * P, CHUNK], BF16, tag="b3")
            b4 = tents.tile([G * P, CHUNK], BF16, tag="b4")
            b5 = tents.tile([G * P, CHUNK], BF16, tag="b5")
            b6 = tents.tile([G * P, CHUNK], BF16, tag="b6")
            b7 = tents.tile([G * P, CHUNK], BF16, tag="b7")
            b8 = tents.tile([G * P, CHUNK], BF16, tag="b8")
            nc.vector.tensor_mul(out=b1, in0=ec, in1=tymn)
            nc.vector.tensor_mul(out=b2, in0=ec, in1=typ)
            nc.vector.tensor_mul(out=b3, in0=ec, in1=txmn)
            nc.vector.tensor_mul(out=b4, in0=ec, in1=txp)
            nc.gpsimd.tensor_mul(out=b5, in0=b1, in1=txmn)
            nc.vector.tensor_mul(out=b6, in0=b1, in1=txp)
            nc.gpsimd.tensor_mul(out=b7, in0=b2, in1=txmn)
            nc.vector.tensor_mul(out=b8, in0=b2, in1=txp)
            basis = [ec, b1, b2, b3, b4, b5, b6, b7, b8]

            psA1 = psum.tile([R1, CHUNK], F32, tag="psa1", bufs=1,
                             name="psA1")
            psA2 = psum.tile([R2, CHUNK], F32, tag="psa2", bufs=1,
                             name="psA2")
            for b in range(NB):
                nc.tensor.matmul(psA1, m1[:, b, :], basis[b],
                                 start=(b == 0), stop=(b == NB - 1))
                nc.tensor.matmul(psA2, m2[:, b, :], basis[b],
                                 start=(b == 0), stop=(b == NB - 1))
            nc.scalar.copy(out=a1[ch], in_=psA1)
            nc.scalar.copy(out=a2[ch], in_=psA2)

        def apply_chunk(ch):
            h0 = ch * 8            # 8 h-rows per 512-pos chunk
            outp = psum4.tile([C, CHUNK], F32, tag="psoacc", bufs=1,
                              name="outp")
            for s in range(NS):
                sy, sx = s // 5 - 2, s % 5 - 2
                if s < NH1:
                    lhsT, arows = ea1[:, s, :], a1[ch]
                else:
                    lhsT, arows = ea2[:, s - NH1, :], a2[ch]
                psBc = psum4.tile([C, CHUNK], F32, tag="psab", bufs=2,
                                  name="psBc")
                nc.tensor.matmul(psBc, lhsT, arows, start=True, stop=True)
                row = 2 + sy + h0
                img = xpj_pad[:, row:row + 8, 2 + sx:2 + sx + W]
                t = temps.tile([C, CHUNK], BF16, tag="t_app")
                mode = APPLY_MODE[s]
                if mode == 'D':      # multiply straight out of PSUM
                    nc.vector.tensor_mul(out=t, in0=psBc, in1=img)
                else:
                    ab = temps.tile([C, CHUNK], BF16,
                                    tag="ab_a" if mode == 'A' else "ab_p")
                    nc.scalar.copy(out=ab, in_=psBc)
                    if mode == 'A':
                        nc.vector.tensor_mul(out=t, in0=ab, in1=img)
                    else:
                        nc.gpsimd.tensor_mul(out=t, in0=ab, in1=img)
                nc.tensor.matmul(outp, w_out, t, start=(s == 0),
                                 stop=(s == NS - 1))

            # evict (+bias), transpose to pos-major via identity matmuls
            fsb = temps.tile([C, CHUNK], BF16, tag="fsb")
            nc.scalar.activation(out=fsb, in_=outp, func=AF.Identity,
                                 bias=b_out_c, scale=1.0)
            pso = psum4.tile([C, 4, C], F32, tag="psab", bufs=2, name="pso")
            for q in range(4):
                nc.tensor.matmul(pso[:, q, :], fsb[:, q * C:(q + 1) * C],
                                 i128, start=True, stop=True)
            osb = temps.tile([C, 4, C], F32, tag="osb")
            nc.scalar.copy(out=osb, in_=pso)
            pos0 = ch * CHUNK
            nc.sync.dma_start(
                out=bass.AP(tensor=io["out"].tensor,
                            offset=io["out"].offset + pos0 * C,
                            ap=[[C, C], [C * C, 4], [1, C]]),
                in_=osb)

        for ch in range(NCH):
            front = build_a(ch)
            if ch > 0:
                apply_chunk(ch - 1)
            build_b(ch, *front)
        apply_chunk(NCH - 1)


# --------------------------------------------------------------------------
# bass module build + public entry point
# --------------------------------------------------------------------------

# Hardware TPB instructions carry exactly ONE sync-wait slot (the
# NEURON_ISA_TPB_EVENTS struct).  Tile can emit several waits on one BIR
# instruction; walrus splits matmult waits across the LDWEIGHTS/MATMULT
# pair, but single-struct ops (Activation, ...) fail codegen with "Too many
# sync wait commands".  Move surplus waits onto standalone same-engine
# EventSemaphore instructions inserted immediately before the offender.
def _wait_cap(ins):
    t = type(ins).__name__
    if t == "InstEventSemaphore":
        return None
    return 1


def _split_surplus_waits(nc):
    import bass_rust
    n = 0
    for bb in nc.m.functions[0].blocks:
        out = []
        for ins in bb.instructions:
            si = getattr(ins, "sync_info", None)
            cap = _wait_cap(ins)
            if si is not None and cap is not None and len(si.on_wait) > cap:
                waits = list(si.on_wait)
                for i, w in enumerate(waits[:-cap]):
                    ev = mybir.InstEventSemaphore(
                        name=f"{ins.name}_xw{i}", ins=[], outs=[])
                    ev.engine = ins.engine
                    ev.sync_info = bass_rust.SyncInfo(on_wait=[w],
                                                     on_update=[])
                    nc.register_instruction(ev)
                    out.append(ev)
                    n += 1
                ins.sync_info = bass_rust.SyncInfo(
                    on_wait=waits[-cap:], on_update=list(si.on_update))
            out.append(ins)
        bb.instructions = out
    return n


_CACHED = {}


def _build_bass():
    if "nc" in _CACHED:
        return _CACHED["nc"]
    nc = bass.Bass()
    io = {}
    specs = {
        "x": (POS, C), "w_in": (C, C), "b_in": (C,), "dwdiag": (C, P, C),
        "dw_b": (C,), "ln_g": (C,), "ln_b": (C,),
        "w_offx": (C, G * P), "w_offy": (C, G * P),
        "b_offx": (G * P,), "b_offy": (G * P,),
        "w_msk": (C, G * P), "b_msk": (G * P,),
        "w_out": (C, C), "b_out": (C,),
        "i128": (C, C),
        "eg": (G * P, G), "egt": (G, G * P),
        "m1": (G * P, NB, R1), "m2": (G * P, NB, R2),
        "ea1": (R1, NH1, C), "ea2": (R2, NH2, C),
        "ones_row": (1, C), "invc_col": (C, 1),
    }
    for name, shape in specs.items():
        dt = BF16 if name in _BF16_INPUTS else F32
        io[name] = nc.dram_tensor(name, list(shape), dt,
                                  kind="ExternalInput").ap()
    io["out"] = nc.dram_tensor("out", [POS, C], F32,
                               kind="ExternalOutput").ap()
    with tile.TileContext(nc) as tc:
        _dcn_tile(tc, io)
    _split_surplus_waits(nc)
    _CACHED["nc"] = nc
    return nc


def make_in_maps(inputs):
    consts = _host_constants(inputs)
    x = np.asarray(inputs["x"], np.float32).reshape(N, POS, C).astype(NPBF)
    base = {
        "w_in": np.asarray(inputs["w_in"], np.float32),
        "b_in": np.asarray(inputs["b_in"], np.float32),
        "dw_b": np.asarray(inputs["dw_b"], np.float32),
        "ln_g": np.asarray(inputs["ln_g"], np.float32),
        "ln_b": np.asarray(inputs["ln_b"], np.float32),
        "w_msk": np.asarray(inputs["w_msk"], np.float32),
        "b_msk": np.asarray(inputs["b_msk"], np.float32),
        "w_out": np.asarray(inputs["w_out"], np.float32),
        "b_out": np.asarray(inputs["b_out"], np.float32),
        **consts,
    }
    base = {k: (v.astype(NPBF) if k in _BF16_INPUTS else v)
            for k, v in base.items()}
    return [{**base, "x": np.ascontiguousarray(x[i])} for i in range(N)]


def kernel(**inputs):
    nc = _build_bass()
    in_maps = make_in_maps(inputs)
    from concourse.bass_utils import run_bass_kernel_spmd
    res = run_bass_kernel_spmd(nc, in_maps, list(range(N)))
    out = np.stack([res.results[i]["out"] for i in range(N)])
    return out.reshape(N, H, W, C).astype(np.float32)


# revision 12
# speedup vs baseline: 1.0608x; 1.0108x over previous
"""DCNv3 (deformable conv v3) forward as a Bass/Tile kernel for Trainium2.

Contract: kernel(**inputs) takes the FULL inputs of reference.setup_inputs()
and returns the FULL (8, 64, 64, 128) output. The batch dim (8) is
data-parallel across 8 NeuronCores; each core runs an identical single-image
program (no collectives).

Algorithm (validated vs the jax reference in numpy, rel err ~4e-6):
  x_proj = x @ w_in + b_in
  x1     = gelu(LN(dwconv3x3(x) + dw_b) * ln_g + ln_b)
  offs   = x1 @ w_off + b_off        (per group g, point p: (ox, oy), |o|<1)
  e      = exp(x1 @ w_msk + b_msk);  m = e / sum_p e
  Bilinear sampling of point p at (h+1+ky+oy, w+1+kx+ox) decomposes into
  per-axis 3-tap tents  t[-1]=relu(-o), t[0]=1-|o|, t[1]=relu(o), so the
  mask-weighted sample sum collapses to a 5x5 shift window:
     out[pos, (g,c)] = sum_{sy,sx in [-2,2]} A[(g,sy,sx), pos] *
                       xproj_pad2[pos + (sy,sx), (g,c)]
  The 9 (dy,dx) tent-product terms are re-expressed in the 9-product basis
     {m, m*tymn, m*typ, m*txmn, m*txp, m*tymn*txmn, m*tymn*txp,
      m*typ*txmn, m*typ*txp},   tmn=min(o,0), tp=max(o,0),
  whose (constant) scatter matrices fold the basis-change coefficients, so
  the on-chip work is 4 one-scalar tensor_scalar tents (4x DVE mode) and 8
  elementwise products instead of 6 slow-path tent ops and 12 products.
  xproj is padded by 2 (inner ring = conv pad inside the sampling grid,
  outer ring = zeros = grid_sample zero padding), making all window reads
  in-bounds with no boundary special cases.
  final  = out @ w_out + b_out

Layout: channel-major [C on partitions, positions on the free axis], so all
channel contractions are natural matmuls. Matmul operands are bf16 (full PE
rate + FWL weight loads); all accumulation (PSUM) is fp32. x^T arrives via
the DMA xbar transpose (no PE/ACT cost). The output projection is fused
into the apply loop: final = sum_s (A_s (*) img_s) @ w_out accumulates
across the 25 shifts in PSUM; per-shift A-broadcast tiles are consumed
three ways to balance engines: evicted to SBUF by ACT, evicted by the
(otherwise idle) Pool engine, or multiplied straight out of PSUM by DVE.
A-build and apply phases share the schedule (two concurrently-open PSUM
pools) so their dependency bubbles overlap.
"""

from contextlib import ExitStack

import ml_dtypes
import numpy as np

import concourse.bass as bass
import concourse.mybir as mybir
import concourse.tile as tile
from concourse._compat import with_exitstack

N, H, W, C, G, K = 8, 64, 64, 128, 8, 3
GC = C // G            # 16
P = K * K              # 9
POS = H * W            # 4096
HP, WP = H + 2, W + 2            # dwconv pad-1 grid (66)
HP2, WP2 = H + 4, W + 4          # sampling pad-2 grid (68)
EPS = 1e-6
NS = 25                          # 5x5 shift window
NH1, NH2 = 13, 12                # A row split: s in [0,13), [13,25)
R1, R2 = G * NH1, G * NH2        # 104, 96 partition rows of the two A halves
NB = 9                           # tent-product basis size
F32 = mybir.dt.float32
BF16 = mybir.dt.bfloat16
NPBF = ml_dtypes.bfloat16

CHUNK = 512                      # free-dim chunk for the build phase
NCH = POS // CHUNK               # 8

AF = mybir.ActivationFunctionType
OP = mybir.AluOpType

# Per-shift handling of the A-broadcast PSUM tile in the apply loop (Pool
# cannot touch PSUM, so every evict is on ACT):
#   'A' = ACT evicts to bf16 SBUF, DVE multiplies at 2x
#   'M' = ACT evicts to bf16 SBUF, Pool multiplies (slow but otherwise idle)
#   'D' = DVE multiplies straight out of PSUM (1x, but no evict at all)
APPLY_MODE = ['A', 'D', 'M', 'D', 'A',
              'D', 'A', 'M', 'D', 'A',
              'D', 'M', 'A', 'D', 'A',
              'D', 'M', 'A', 'D', 'A',
              'D', 'M', 'A', 'D', 'D']

# basis-change coefficients: d-term (dy_idx, dx_idx) -> {basis index: coeff}
# with stored tents tmn=min(o,0), tp=max(o,0) and actual taps
# t[-1]=-tmn, t[0]=1+tmn-tp, t[1]=tp.
_COEFF = {
    (0, 0): {5: 1.0},
    (0, 1): {1: -1.0, 5: -1.0, 6: 1.0},
    (0, 2): {6: -1.0},
    (1, 0): {3: -1.0, 5: -1.0, 7: 1.0},
    (1, 1): {0: 1.0, 1: 1.0, 2: -1.0, 3: 1.0, 4: -1.0,
             5: 1.0, 6: -1.0, 7: -1.0, 8: 1.0},
    (1, 2): {4: 1.0, 6: 1.0, 8: -1.0},
    (2, 0): {7: -1.0},
    (2, 1): {2: 1.0, 7: 1.0, 8: -1.0},
    (2, 2): {8: 1.0},
}


# --------------------------------------------------------------------------
# host-side constant matrices
# --------------------------------------------------------------------------

def _host_constants(inputs):
    dw_w = np.asarray(inputs["dw_w"], np.float32)        # (3,3,1,C) [ky,kx]
    w_off = np.asarray(inputs["w_off"], np.float32)      # (C, G*P*2)
    b_off = np.asarray(inputs["b_off"], np.float32)      # (G*P*2,)

    # depthwise weights as 9 diagonal matrices, c-major: [c_row, s, c_col]
    dwdiag = np.zeros((C, P, C), np.float32)
    for s in range(P):
        ky, kx = s // 3, s % 3
        dwdiag[np.arange(C), s, np.arange(C)] = dw_w[ky, kx, 0]

    w_offx = np.ascontiguousarray(w_off[:, 0::2])        # (C, 72)
    w_offy = np.ascontiguousarray(w_off[:, 1::2])
    b_offx = np.ascontiguousarray(b_off[0::2])           # (72,)
    b_offy = np.ascontiguousarray(b_off[1::2])

    # block-ones matrix: one matmul produces the group sums broadcast to
    # all 72 (g,p) rows at once (softmax denominator)
    egg = np.zeros((G * P, G * P), np.float32)
    for g in range(G):
        egg[g * P:(g + 1) * P, g * P:(g + 1) * P] = 1.0

    # A-scatter matrices over the 9-product basis: basis term b of point
    # (g,p) lands in A row (g, s), s = (p%3 + dy)*5 + (p//3 + dx) (x-major
    # p!), weighted by the basis-change coefficient of d-term (dy,dx).
    m1 = np.zeros((G * P, NB, R1), np.float32)
    m2 = np.zeros((G * P, NB, R2), np.float32)
    for (dy, dx), cs in _COEFF.items():
        for g in range(G):
            for p in range(P):
                s = (p % 3 + dy) * 5 + (p // 3 + dx)
                for b, coef in cs.items():
                    if s < NH1:
                        m1[g * P + p, b, g * NH1 + s] += coef
                    else:
                        m2[g * P + p, b, g * NH2 + (s - NH1)] += coef

    # A-broadcast matrices: A row (g, s) -> output row (g*GC + c)
    ea1 = np.zeros((R1, NH1, C), np.float32)
    ea2 = np.zeros((R2, NH2, C), np.float32)
    for g in range(G):
        for sl in range(NH1):
            ea1[g * NH1 + sl, sl, g * GC:(g + 1) * GC] = 1.0
        for sl in range(NH2):
            ea2[g * NH2 + sl, sl, g * GC:(g + 1) * GC] = 1.0

    ones_row = np.ones((1, C), np.float32)
    invc_col = np.full((C, 1), 1.0 / C, np.float32)
    i128 = np.eye(C, dtype=np.float32)

    return {
        "dwdiag": dwdiag, "w_offx": w_offx, "w_offy": w_offy,
        "b_offx": b_offx, "b_offy": b_offy,
        "egg": egg,
        "m1": m1, "m2": m2, "ea1": ea1, "ea2": ea2,
        "ones_row": ones_row, "invc_col": invc_col, "i128": i128,
    }


# names of DRAM inputs delivered as bf16 (matmul operands; x additionally
# rides the DMA xbar transpose, which needs a 2-byte dtype).
_BF16_INPUTS = {
    "x", "w_in", "dwdiag", "w_offx", "w_offy", "w_msk", "w_out",
    "egg", "m1", "m2", "ea1", "ea2", "ones_row", "invc_col",
    "i128",
}


# --------------------------------------------------------------------------
# the per-core Tile program
# --------------------------------------------------------------------------

@with_exitstack
def _dcn_tile(ctx: ExitStack, tc: tile.TileContext, io: dict):
    nc = tc.nc
    ctx.enter_context(nc.allow_low_precision(
        reason="bf16 matmul operands; accumulation stays fp32 in PSUM and "
               "in the fp32 sampling accumulator"))

    persist = ctx.enter_context(tc.tile_pool(name="persist", bufs=1))
    temps = ctx.enter_context(tc.tile_pool(name="temps", bufs=3))
    tents = ctx.enter_context(tc.tile_pool(name="tents", bufs=2))

    # ---- load weights / constants -------------------------------------
    def load(name, shape, col=False):
        dt = BF16 if name in _BF16_INPUTS else F32
        t = persist.tile(shape, dt, tag=f"w_{name}")
        src = io[name]
        if col:  # DRAM vector (n,) -> SBUF [n, 1]
            src = bass.AP(tensor=src.tensor, offset=src.offset,
                          ap=[[1, shape[0]], [1, 1]])
        nc.sync.dma_start(out=t, in_=src)
        return t

    w_in = load("w_in", [C, C])
    w_out = load("w_out", [C, C])
    w_offx = load("w_offx", [C, G * P])
    w_offy = load("w_offy", [C, G * P])
    w_msk = load("w_msk", [C, G * P])
    dwdiag = load("dwdiag", [C, P, C])          # [c_row, s, c_col]
    egg = load("egg", [G * P, G * P])
    m1 = load("m1", [G * P, NB, R1])
    m2 = load("m2", [G * P, NB, R2])
    ea1 = load("ea1", [R1, NH1, C])
    ea2 = load("ea2", [R2, NH2, C])
    ones_row = load("ones_row", [1, C])
    invc_col = load("invc_col", [C, 1])
    i128 = load("i128", [C, C])
    b_in = load("b_in", [C, 1], col=True)
    dw_b = load("dw_b", [C, 1], col=True)
    ln_g = load("ln_g", [C, 1], col=True)
    ln_b = load("ln_b", [C, 1], col=True)
    b_out_c = load("b_out", [C, 1], col=True)
    b_offx = load("b_offx", [G * P, 1], col=True)
    b_offy = load("b_offy", [G * P, 1], col=True)
    b_msk = load("b_msk", [G * P, 1], col=True)

    eps1 = persist.tile([1, 1], F32)
    nc.vector.memset(eps1, EPS)

    # ---- persistent activations ---------------------------------------
    xt_pad = persist.tile([C, HP, WP], BF16)     # x^T, conv-padded (66x66)
    xpj_pad = persist.tile([C, HP2, WP2], BF16)  # x_proj^T, pad-2 (68x68)
    a1 = [persist.tile([R1, CHUNK], BF16, tag=f"a1_{i}", name=f"a1_{i}")
          for i in range(NCH)]
    a2 = [persist.tile([R2, CHUNK], BF16, tag=f"a2_{i}", name=f"a2_{i}")
          for i in range(NCH)]

    # only the pad rings need zeroing; the interiors are fully overwritten
    nc.vector.memset(xt_pad[:, 0:1, :], 0.0)
    nc.vector.memset(xt_pad[:, HP - 1:HP, :], 0.0)
    nc.vector.memset(xt_pad[:, 1:HP - 1, 0:1], 0.0)
    nc.vector.memset(xt_pad[:, 1:HP - 1, WP - 1:WP], 0.0)
    nc.vector.memset(xpj_pad[:, 0:2, :], 0.0)
    nc.vector.memset(xpj_pad[:, HP2 - 2:HP2, :], 0.0)
    nc.vector.memset(xpj_pad[:, 2:HP2 - 2, 0:2], 0.0)
    nc.vector.memset(xpj_pad[:, 2:HP2 - 2, WP2 - 2:WP2], 0.0)

    # ---- stage 1: x^T via the DMA xbar transpose ----------------------
    # The interpreter's transpose semantics only match AP-linear order for
    # 2D outputs, so land in a contiguous tile and restride with one DVE
    # copy (4x mode) into the padded grid.
    xt_flat = persist.tile([C, POS], BF16)
    nc.sync.dma_start(out=xt_flat, in_=io["x"], transpose=True)
    nc.vector.tensor_copy(out=xt_pad[:, 1:1 + H, 1:1 + W], in_=xt_flat)

    # ---- stage 2: x_proj into the pad-2 grid --------------------------
    with tc.tile_pool(name="ps_s12", bufs=2, space="PSUM") as psum:
        for ch in range(NCH):        # chunk = 8 h-rows
            h0 = ch * 8
            rhs = xt_pad[:, 1 + h0:1 + h0 + 8, 1:1 + W]
            ps = psum.tile([C, CHUNK], F32, tag="ps_proj")
            nc.tensor.matmul(ps, w_in, rhs, start=True, stop=True)
            nc.scalar.activation(
                out=xpj_pad[:, 2 + h0:2 + h0 + 8, 2:2 + W],
                in_=ps.rearrange("c (a b) -> c a b", b=W),
                func=AF.Identity, bias=b_in, scale=1.0)

    # ---- stages 3+4, interleaved ---------------------------------------
    # Two PSUM pools, both open across the whole fused phase so the
    # scheduler can interleave A-building and applying freely.
    with tc.tile_pool(name="ps3", bufs=1, space="PSUM") as psum, \
            tc.tile_pool(name="ps4", bufs=1, space="PSUM") as psum4:

        def build_a(ch):
            """dwconv + the [1,512] LayerNorm stats sub-chain.  Emitted
            BEFORE apply(ch-1) so the long serial LN chain percolates
            through ACT/DVE/Pool while PE grinds the apply matmuls."""
            h0 = ch * 8
            # depthwise conv via 9 diagonal matmuls
            psA = psum.tile([C, CHUNK], F32, tag="psdw", bufs=1, name="psA")
            for s in range(P):
                ky, kx = s // 3, s % 3
                rhs = xt_pad[:, h0 + ky:h0 + ky + 8, kx:kx + W]
                nc.tensor.matmul(psA, dwdiag[:, s, :], rhs,
                                 start=(s == 0), stop=(s == P - 1))
            x1c = temps.tile([C, CHUNK], BF16, tag="x1c")
            nc.scalar.activation(out=x1c, in_=psA, func=AF.Identity,
                                 bias=dw_b, scale=1.0)

            # LayerNorm stats over channels (partition dim) via 1/C cols;
            # x1sq on the Pool engine so it doesn't queue behind the
            # previous chunk's apply muls on DVE.
            x1sq = temps.tile([C, CHUNK], BF16, tag="scr")
            nc.gpsimd.tensor_mul(out=x1sq, in0=x1c, in1=x1c)
            pstat = psum.tile([33, CHUNK], F32, tag="psdw", bufs=1,
                              name="pstat")
            nc.tensor.matmul(pstat[0:1, :], invc_col, x1c, start=True,
                             stop=True)
            mean_r = temps.tile([1, CHUNK], BF16, tag="mean_r")
            nc.scalar.copy(out=mean_r, in_=pstat[0:1, :])
            nc.tensor.matmul(pstat[32:33, :], invc_col, x1sq, start=True,
                             stop=True)
            tmp_r = temps.tile([1, CHUNK], F32, tag="tmp_r")
            nc.scalar.activation(out=tmp_r, in_=mean_r, func=AF.Square)
            nc.vector.tensor_sub(out=tmp_r, in0=pstat[32:33, :], in1=tmp_r)
            nc.scalar.activation(out=tmp_r, in_=tmp_r, func=AF.Sqrt,
                                 bias=eps1, scale=1.0)            # std
            rstd_r = temps.tile([1, CHUNK], BF16, tag="rstd_r")
            nc.vector.reciprocal(out=rstd_r, in_=tmp_r)           # rstd
            mrs_r = temps.tile([1, CHUNK], BF16, tag="mrs_r")
            nc.vector.tensor_mul(out=mrs_r, in0=mean_r, in1=rstd_r)
            return x1c, rstd_r, mrs_r

        def build_b(ch, x1c, rstd_r, mrs_r):
            h0 = ch * 8
            psR = psum.tile([C, CHUNK], F32, tag="psrm", bufs=1, name="psR")
            nc.tensor.matmul(psR, ones_row, rstd_r, start=True, stop=True)
            zc = temps.tile([C, CHUNK], F32, tag="scr2")
            nc.vector.tensor_mul(out=zc, in0=x1c, in1=psR)
            psM = psum.tile([C, CHUNK], F32, tag="psoff", bufs=2, name="psM")
            nc.tensor.matmul(psM, ones_row, mrs_r, start=True, stop=True)
            nc.vector.tensor_sub(out=zc, in0=zc, in1=psM)
            x1gc = temps.tile([C, CHUNK], BF16, tag="x1gc")
            nc.scalar.activation(out=x1gc, in_=zc, func=AF.Gelu,
                                 bias=ln_b, scale=ln_g)

            # offset projections; tents as one-scalar tensor_scalar ops on
            # bf16 SBUF evictions (4x DVE mode)
            psX = psum.tile([G * P, CHUNK], F32, tag="psoff", bufs=2,
                            name="psX")
            nc.tensor.matmul(psX, w_offx, x1gc, start=True, stop=True)
            oxs = tents.tile([G * P, CHUNK], BF16, tag="oxs")
            nc.scalar.activation(out=oxs, in_=psX, func=AF.Identity,
                                 bias=b_offx, scale=1.0)
            txmn = tents.tile([G * P, CHUNK], BF16, tag="txmn")
            txp = tents.tile([G * P, CHUNK], BF16, tag="txp")
            nc.vector.tensor_scalar(out=txmn, in0=oxs, scalar1=0.0,
                                    scalar2=None, op0=OP.min)
            nc.vector.tensor_scalar(out=txp, in0=oxs, scalar1=0.0,
                                    scalar2=None, op0=OP.max)
            psY = psum.tile([G * P, CHUNK], F32, tag="psoff", bufs=2,
                            name="psY")
            nc.tensor.matmul(psY, w_offy, x1gc, start=True, stop=True)
            oys = tents.tile([G * P, CHUNK], BF16, tag="oys")
            nc.scalar.activation(out=oys, in_=psY, func=AF.Identity,
                                 bias=b_offy, scale=1.0)
            tymn = tents.tile([G * P, CHUNK], BF16, tag="tymn")
            typ = tents.tile([G * P, CHUNK], BF16, tag="typ")
            nc.vector.tensor_scalar(out=tymn, in0=oys, scalar1=0.0,
                                    scalar2=None, op0=OP.min)
            nc.vector.tensor_scalar(out=typ, in0=oys, scalar1=0.0,
                                    scalar2=None, op0=OP.max)

            # normalized mask: e_n = exp(l + b) / group sum
            psE = psum.tile([G * P, CHUNK], F32, tag="psoff", bufs=2,
                            name="psE")
            nc.tensor.matmul(psE, w_msk, x1gc, start=True, stop=True)
            ec = temps.tile([G * P, CHUNK], BF16, tag="ec")
            nc.scalar.activation(out=ec, in_=psE, func=AF.Exp,
                                 bias=b_msk, scale=1.0)
            psB = psum.tile([G * P, CHUNK], F32, tag="psoff", bufs=2,
                            name="psB")
            nc.tensor.matmul(psB, egg, ec, start=True, stop=True)
            rec_b = temps.tile([G * P, CHUNK], BF16, tag="rec_b")
            nc.vector.reciprocal(out=rec_b, in_=psB)
            nc.vector.tensor_mul(out=ec, in0=ec, in1=rec_b)

            # 8 basis products (all-bf16 SBUF tensor_tensor, 2x mode)
            b1 = tents.tile([G * P, CHUNK], BF16, tag="b1")
            b2 = tents.tile([G * P, CHUNK], BF16, tag="b2")
            b3 = tents.tile([G * P, CHUNK], BF16, tag="b3")
            b4 = tents.tile([G * P, CHUNK], BF16, tag="b4")
            b5 = tents.tile([G * P, CHUNK], BF16, tag="b5")
            b6 = tents.tile([G * P, CHUNK], BF16, tag="b6")
            b7 = tents.tile([G * P, CHUNK], BF16, tag="b7")
            b8 = tents.tile([G * P, CHUNK], BF16, tag="b8")
            nc.vector.tensor_mul(out=b1, in0=ec, in1=tymn)
            nc.vector.tensor_mul(out=b2, in0=ec, in1=typ)
            nc.vector.tensor_mul(out=b3, in0=ec, in1=txmn)
            nc.vector.tensor_mul(out=b4, in0=ec, in1=txp)
            nc.gpsimd.tensor_mul(out=b5, in0=b1, in1=txmn)
            nc.vector.tensor_mul(out=b6, in0=b1, in1=txp)
            nc.gpsimd.tensor_mul(out=b7, in0=b2, in1=txmn)
            nc.vector.tensor_mul(out=b8, in0=b2, in1=txp)
            basis = [ec, b1, b2, b3, b4, b5, b6, b7, b8]

            psA1 = psum.tile([R1, CHUNK], F32, tag="psa", bufs=1,
                             name="psA1")
            for b in range(NB):
                nc.tensor.matmul(psA1, m1[:, b, :], basis[b],
                                 start=(b == 0), stop=(b == NB - 1))
            nc.scalar.copy(out=a1[ch], in_=psA1)
            psA2 = psum.tile([R2, CHUNK], F32, tag="psa", bufs=1,
                             name="psA2")
            for b in range(NB):
                nc.tensor.matmul(psA2, m2[:, b, :], basis[b],
                                 start=(b == 0), stop=(b == NB - 1))
            nc.scalar.copy(out=a2[ch], in_=psA2)

        def apply_chunk(ch):
            h0 = ch * 8            # 8 h-rows per 512-pos chunk
            outp = psum4.tile([C, CHUNK], F32, tag="psoacc", bufs=1,
                              name="outp")
            for s in range(NS):
                sy, sx = s // 5 - 2, s % 5 - 2
                if s < NH1:
                    lhsT, arows = ea1[:, s, :], a1[ch]
                else:
                    lhsT, arows = ea2[:, s - NH1, :], a2[ch]
                psBc = psum4.tile([C, CHUNK], F32, tag="psab", bufs=2,
                                  name="psBc")
                nc.tensor.matmul(psBc, lhsT, arows, start=True, stop=True)
                row = 2 + sy + h0
                img = xpj_pad[:, row:row + 8, 2 + sx:2 + sx + W]
                t = temps.tile([C, CHUNK], BF16, tag="t_app")
                mode = APPLY_MODE[s]
                if mode == 'D':      # multiply straight out of PSUM
                    nc.vector.tensor_mul(out=t, in0=psBc, in1=img)
                else:
                    ab = temps.tile([C, CHUNK], BF16,
                                    tag="ab_a" if mode == 'A' else "ab_p")
                    nc.scalar.copy(out=ab, in_=psBc)
                    if mode == 'A':
                        nc.vector.tensor_mul(out=t, in0=ab, in1=img)
                    else:
                        nc.gpsimd.tensor_mul(out=t, in0=ab, in1=img)
                nc.tensor.matmul(outp, w_out, t, start=(s == 0),
                                 stop=(s == NS - 1))

            # evict (+bias), transpose to pos-major via identity matmuls
            fsb = temps.tile([C, CHUNK], BF16, tag="fsb")
            nc.scalar.activation(out=fsb, in_=outp, func=AF.Identity,
                                 bias=b_out_c, scale=1.0)
            pso = psum4.tile([C, 4, C], F32, tag="psab", bufs=2, name="pso")
            for q in range(4):
                nc.tensor.matmul(pso[:, q, :], fsb[:, q * C:(q + 1) * C],
                                 i128, start=True, stop=True)
            osb = temps.tile([C, 4, C], F32, tag="osb")
            nc.scalar.copy(out=osb, in_=pso)
            pos0 = ch * CHUNK
            nc.sync.dma_start(
                out=bass.AP(tensor=io["out"].tensor,
                            offset=io["out"].offset + pos0 * C,
                            ap=[[C, C], [C * C, 4], [1, C]]),
                in_=osb)

        for ch in range(NCH):
            front = build_a(ch)
            if ch > 0:
                apply_chunk(ch - 1)
            build_b(ch, *front)
        apply_chunk(NCH - 1)


# --------------------------------------------------------------------------
# bass module build + public entry point
# --------------------------------------------------------------------------

# Hardware TPB instructions carry exactly ONE sync-wait slot (the
# NEURON_ISA_TPB_EVENTS struct).  Tile can emit several waits on one BIR
# instruction; walrus splits matmult waits across the LDWEIGHTS/MATMULT
# pair, but single-struct ops (Activation, ...) fail codegen with "Too many
# sync wait commands".  Move surplus waits onto standalone same-engine
# EventSemaphore instructions inserted immediately before the offender.
def _wait_cap(ins):
    t = type(ins).__name__
    if t == "InstEventSemaphore":
        return None
    return 1


def _split_surplus_waits(nc):
    import bass_rust
    n = 0
    for bb in nc.m.functions[0].blocks:
        out = []
        for ins in bb.instructions:
            si = getattr(ins, "sync_info", None)
            cap = _wait_cap(ins)
            if si is not None and cap is not None and len(si.on_wait) > cap:
                waits = list(si.on_wait)
                for i, w in enumerate(waits[:-cap]):
                    ev = mybir.InstEventSemaphore(
                        name=f"{ins.name}_xw{i}", ins=[], outs=[])
                    ev.engine = ins.engine
                    ev.sync_info = bass_rust.SyncInfo(on_wait=[w],
                                                     on_update=[])
                    nc.register_instruction(ev)
                    out.append(ev)
                    n += 1
                ins.sync_info = bass_rust.SyncInfo(
                    on_wait=waits[-cap:], on_update=list(si.on_update))
            out.append(ins)
        bb.instructions = out
    return n


_CACHED = {}


def _build_bass():
    if "nc" in _CACHED:
        return _CACHED["nc"]
    nc = bass.Bass()
    io = {}
    specs = {
        "x": (POS, C), "w_in": (C, C), "b_in": (C,), "dwdiag": (C, P, C),
        "dw_b": (C,), "ln_g": (C,), "ln_b": (C,),
        "w_offx": (C, G * P), "w_offy": (C, G * P),
        "b_offx": (G * P,), "b_offy": (G * P,),
        "w_msk": (C, G * P), "b_msk": (G * P,),
        "w_out": (C, C), "b_out": (C,),
        "i128": (C, C),
        "egg": (G * P, G * P),
        "m1": (G * P, NB, R1), "m2": (G * P, NB, R2),
        "ea1": (R1, NH1, C), "ea2": (R2, NH2, C),
        "ones_row": (1, C), "invc_col": (C, 1),
    }
    for name, shape in specs.items():
        dt = BF16 if name in _BF16_INPUTS else F32
        io[name] = nc.dram_tensor(name, list(shape), dt,
                                  kind="ExternalInput").ap()
    io["out"] = nc.dram_tensor("out", [POS, C], F32,
                               kind="ExternalOutput").ap()
    with tile.TileContext(nc) as tc:
        _dcn_tile(tc, io)
    _split_surplus_waits(nc)
    _CACHED["nc"] = nc
    return nc


def make_in_maps(inputs):
    consts = _host_constants(inputs)
    x = np.asarray(inputs["x"], np.float32).reshape(N, POS, C).astype(NPBF)
    base = {
        "w_in": np.asarray(inputs["w_in"], np.float32),
        "b_in": np.asarray(inputs["b_in"], np.float32),
        "dw_b": np.asarray(inputs["dw_b"], np.float32),
        "ln_g": np.asarray(inputs["ln_g"], np.float32),
        "ln_b": np.asarray(inputs["ln_b"], np.float32),
        "w_msk": np.asarray(inputs["w_msk"], np.float32),
        "b_msk": np.asarray(inputs["b_msk"], np.float32),
        "w_out": np.asarray(inputs["w_out"], np.float32),
        "b_out": np.asarray(inputs["b_out"], np.float32),
        **consts,
    }
    base = {k: (v.astype(NPBF) if k in _BF16_INPUTS else v)
            for k, v in base.items()}
    return [{**base, "x": np.ascontiguousarray(x[i])} for i in range(N)]


def kernel(**inputs):
    nc = _build_bass()
    in_maps = make_in_maps(inputs)
    from concourse.bass_utils import run_bass_kernel_spmd
    res = run_bass_kernel_spmd(nc, in_maps, list(range(N)))
    out = np.stack([res.results[i]["out"] for i in range(N)])
    return out.reshape(N, H, W, C).astype(np.float32)
